# revision 15
# baseline (speedup 1.0000x reference)
"""Multi-head self-attention (RoPE, causal) Trainium2 Bass kernel.

Sharding: tensor-parallel over heads. 16 heads / 8 cores = 2 heads per core.
Each core computes Q/K/V projections for its 2 heads, causal attention, and a
partial output projection against its 256-column slice of Wo. The host sums
the 8 partial [S, D] outputs.

All matmuls run in bf16 with fp32 PSUM accumulation. Softmax skips the max
subtraction (scores are O(1) for this problem family; exp stays in fp32
range). The attention*V phase is computed transposed — V tiles are the
stationary operand and P^T streams 512-wide q-chunks — so the PE streams full
free dims instead of 129-wide LDW-bound matmuls, and the output lands
directly in the [dh, q] layout the output projection wants (no transposes).
The softmax denominator comes from an f32 running sum of P^T tiles on the
vector engine, reduced across partitions by gpsimd.partition_all_reduce.
RoPE's even/odd interleave is folded into a row permutation of Wq/Wk, making
the on-device rotation a contiguous rotate-half.
"""
import sys

sys.path.insert(0, "/opt/trn_rl_repo")

import numpy as np
import ml_dtypes

import concourse.bass as bass  # noqa: F401  (registers AP machinery)
import concourse.tile as tile
from concourse import bacc, bass_isa, mybir
from concourse import bass_utils

BF16 = ml_dtypes.bfloat16
S = 4096
D = 2048
DH = 128
N_CORES = 8
HPC = 2  # heads per core
PW = 512  # projection s-window
QW = 1024  # attention q-window
N_QW = S // QW  # 4
SUBS = QW // 128  # 8 q-subtiles per window
N_KT = S // 128  # 32 k-tiles
INV_SQRT_DH = float(1.0 / np.sqrt(128.0))

_CACHE = {}


def _build(reps=1):
    fp32 = mybir.dt.float32
    bf16 = mybir.dt.bfloat16

    nc = bacc.Bacc("TRN2", target_bir_lowering=False, debug=False,
                   num_devices=N_CORES)
    xT_d = nc.dram_tensor("xTw", [128, S * 16], bf16,
                          kind="ExternalInput").ap()
    wq_d = nc.dram_tensor("wqT", [128, 16 * HPC * DH], bf16,
                          kind="ExternalInput").ap()
    wk_d = nc.dram_tensor("wkT", [128, 16 * HPC * DH], bf16,
                          kind="ExternalInput").ap()
    wv_d = nc.dram_tensor("wvT", [128, 16 * HPC * DH], bf16,
                          kind="ExternalInput").ap()
    wo_d = nc.dram_tensor("woT", [HPC * DH, D], bf16, kind="ExternalInput").ap()
    cos_d = nc.dram_tensor("cosF", [128, S], bf16, kind="ExternalInput").ap()
    sin_d = nc.dram_tensor("sinX", [128, S], bf16, kind="ExternalInput").ap()
    swp_d = nc.dram_tensor("swp", [128, 128], bf16, kind="ExternalInput").ap()
    mask_d = nc.dram_tensor("mask", [128, 128], bf16, kind="ExternalInput").ap()
    out_d = nc.dram_tensor("out", [S, D], bf16, kind="ExternalOutput").ap()

    xT_r = xT_d.rearrange("p (w t s) -> p w t s", w=S // PW,
                          t=16)               # [128, 8, 16, 512]
    wq_r = wq_d.rearrange("p (t m) -> p t m", t=16)    # [128, 16, 256]
    wk_r = wk_d.rearrange("p (t m) -> p t m", t=16)
    wv_r = wv_d.rearrange("p (t m) -> p t m", t=16)

    EXP = mybir.ActivationFunctionType.Exp

    from contextlib import ExitStack

    def emit_body(tc):
        with tc.tile_pool(name="persist", bufs=1) as pp, \
             tc.tile_pool(name="ropet", bufs=1) as rtp:
            # ---- persistent tiles + initial loads (spread across engines)
            qt = [pp.tile([128, S], bf16, tag=f"qt{h}", name=f"qt{h}")
                  for h in range(HPC)]
            kt = [pp.tile([128, S], bf16, tag=f"kt{h}", name=f"kt{h}")
                  for h in range(HPC)]
            v_sb = pp.tile([128, N_KT, HPC * DH], bf16, tag="v")
            oc = [[pp.tile([128, QW], bf16, tag=f"oc{h}w{w}", name=f"oc{h}w{w}")
                   for w in range(N_QW)] for h in range(HPC)]
            cos_sb = pp.tile([128, S], bf16, tag="cos")
            sin_sb = pp.tile([128, S], bf16, tag="sin")
            mask_sb = pp.tile([128, 128], bf16, tag="mask")
            swp_sb = pp.tile([128, 128], bf16, tag="swp")

            stA = ExitStack()
            psA = stA.enter_context(
                tc.tile_pool(name="psA", bufs=2, space="PSUM"))
            wqkp = stA.enter_context(tc.tile_pool(name="wqk", bufs=1))
            wq_sb = wqkp.tile([128, 16, HPC * DH], bf16, tag="wq")
            wk_sb = wqkp.tile([128, 16, HPC * DH], bf16, tag="wk")
            stV = ExitStack()
            psV = stV.enter_context(
                tc.tile_pool(name="psV", bufs=2, space="PSUM"))
            wvp = stV.enter_context(tc.tile_pool(name="wvp", bufs=1))
            xw1p = stV.enter_context(tc.tile_pool(name="xw1", bufs=3))
            wv_sb = wvp.tile([128, 16, HPC * DH], bf16, tag="wv")

            # critical-path loads first on the two HWDGE queues (sync,
            # scalar); background preloads ride the slower gpsimd SWDGE
            nc.sync.dma_start(out=wq_sb, in_=wq_r)
            nc.gpsimd.dma_start(out=wk_sb, in_=wk_r)
            nc.scalar.dma_start(out=wv_sb, in_=wv_r)
            nc.scalar.dma_start(out=swp_sb, in_=swp_d)
            wo_sb = [pp.tile([128, D], bf16, tag=f"wo{t}", name=f"wo{t}")
                     for t in range(HPC)]

            def project_qk(h, xw, sl, pw):
                hs = slice(h * DH, (h + 1) * DH)
                for wsb, dest in ((wq_sb, qt[h]), (wk_sb, kt[h])):
                    ps = psA.tile([128, pw], fp32, tag="qk", name="ps",
                                  padded_shape=[128, PW])
                    for t in range(16):
                        nc.tensor.matmul(ps, wsb[:, t, hs], xw[:, t, :],
                                         start=(t == 0), stop=(t == 15))
                    if h == 0:
                        nc.scalar.copy(out=dest[:, sl], in_=ps)
                    else:
                        nc.vector.tensor_copy(dest[:, sl], ps)
                    # rope in place: dest = dest*cosF + swap(dest)*[-sin;sin]
                    dsl = dest[:, sl]
                    swp = psA.tile([128, pw], fp32, tag="qk", bufs=2,
                                   name="swp", padded_shape=[128, PW])
                    nc.tensor.matmul(swp, swp_sb, dsl, start=True, stop=True)
                    m1 = rtp.tile([128, pw], bf16, tag="m1", name="m1",
                                  padded_shape=[128, PW])
                    m2 = rtp.tile([128, pw], bf16, tag="m2", name="m2",
                                  padded_shape=[128, PW])
                    nc.vector.tensor_mul(m1, dsl, cos_sb[:, sl])
                    nc.vector.tensor_mul(m2, swp, sin_sb[:, sl])
                    nc.vector.tensor_add(dsl, m1, m2)

            # ---------------- A1: head-0 Q/K + all V ----------------
            for w in range(S // PW):
                sl = slice(w * PW, (w + 1) * PW)
                xw = xw1p.tile([128, 16, PW], bf16, tag="xw")
                nc.sync.dma_start(out=xw[:, 0:8, :], in_=xT_r[:, w, 0:8, :])
                nc.gpsimd.dma_start(out=xw[:, 8:16, :],
                                    in_=xT_r[:, w, 8:16, :])
                if w == 0:
                    nc.scalar.dma_start(out=cos_sb, in_=cos_d)
                elif w == 1:
                    nc.scalar.dma_start(out=sin_sb, in_=sin_d)
                elif w == 2:
                    nc.scalar.dma_start(out=mask_sb, in_=mask_d)
                    for t in range(HPC):
                        nc.scalar.dma_start(
                            out=wo_sb[t], in_=wo_d[t * 128:(t + 1) * 128, :])
                project_qk(0, xw, sl, PW)
                for sub in range(PW // 128):
                    st = w * (PW // 128) + sub
                    ssl = slice(sub * 128, (sub + 1) * 128)
                    pv = psV.tile([128, HPC * DH], fp32, tag="v")
                    for t in range(16):
                        nc.tensor.matmul(pv, xw[:, t, ssl], wv_sb[:, t, :],
                                         start=(t == 0), stop=(t == 15))
                    nc.scalar.copy(out=v_sb[:, st, :], in_=pv)
            stV.close()

            # ---------------- B machinery ----------------
            stB = ExitStack()
            ptp = stB.enter_context(tc.tile_pool(name="pt", bufs=1))
            psmp = stB.enter_context(tc.tile_pool(name="psm", bufs=1))
            rcpp = stB.enter_context(tc.tile_pool(name="rcp", bufs=2))
            pssc = stB.enter_context(
                tc.tile_pool(name="pssc", bufs=2, space="PSUM"))
            psag = stB.enter_context(
                tc.tile_pool(name="psag", bufs=2, space="PSUM"))

            def attn_window(h, w, after_chunk=None):
                hs2 = slice(h * DH, (h + 1) * DH)
                q0 = w * QW
                n_j = SUBS * w + SUBS
                par = [2 if w > 0 else 1, 2]  # accumulators in use per chunk
                psum = [[psmp.tile([128, 512], bf16, tag=f"psum{c2}p{p}",
                                   name=f"psum{c2}p{p}")
                         for p in range(par[c2])] for c2 in range(2)]
                nacc = [0, 0]  # contributions so far per chunk
                pts = []
                for j in range(n_j):
                    ksl = slice(j * 128, (j + 1) * 128)
                    c = j - SUBS * w  # >= 0 -> diagonal strip
                    lo = max(0, c) * 128
                    sc = pssc.tile([128, QW], fp32, tag="sc", name="sc")
                    if lo < 512:
                        nc.tensor.matmul(sc[:, lo:512], kt[h][:, ksl],
                                         qt[h][:, q0 + lo:q0 + 512],
                                         start=True, stop=True)
                        nc.tensor.matmul(sc[:, 512:1024], kt[h][:, ksl],
                                         qt[h][:, q0 + 512:q0 + 1024],
                                         start=True, stop=True)
                    else:
                        nc.tensor.matmul(sc[:, lo:1024], kt[h][:, ksl],
                                         qt[h][:, q0 + lo:q0 + 1024],
                                         start=True, stop=True)
                    pt = ptp.tile([128, QW], bf16, tag=f"pt{j}",
                                  name=f"pt{j}")
                    nc.scalar.activation(pt[:, lo:], sc[:, lo:], EXP,
                                         scale=INV_SQRT_DH)
                    if c >= 0:
                        csl = slice(c * 128, (c + 1) * 128)
                        nc.vector.tensor_mul(pt[:, csl], pt[:, csl], mask_sb)
                    # running bf16 denominator sums: per 512-chunk (so
                    # chunk-0's D closes 4 exps early) and parity-split to
                    # halve the serial DVE chain; valid columns only
                    for c2 in range(2):
                        a = max(lo - c2 * 512, 0)
                        if a >= 512:
                            continue
                        acc = psum[c2][nacc[c2] % par[c2]]
                        srcp = pt[:, c2 * 512 + a:(c2 + 1) * 512]
                        if nacc[c2] < par[c2]:
                            assert a == 0 or par[c2] == 1 or nacc[c2] == 0
                            if a == 0:
                                nc.vector.tensor_copy(
                                    acc, pt[:, c2 * 512:(c2 + 1) * 512])
                            else:
                                nc.vector.tensor_copy(acc[:, a:], srcp)
                                nc.vector.memset(acc[:, 0:a], 0.0)
                        else:
                            nc.vector.tensor_add(acc[:, a:], acc[:, a:],
                                                 srcp)
                        nacc[c2] += 1
                    pts.append(pt)
                # denominator: all-partitions sum (f32 internally) -> recip
                rc = rcpp.tile([128, QW], fp32, tag="rc", name="rc")
                for c2 in range(2):
                    cl = slice(c2 * 512, (c2 + 1) * 512)
                    if par[c2] == 2:
                        nc.vector.tensor_add(psum[c2][0], psum[c2][0],
                                             psum[c2][1])
                    nc.gpsimd.partition_all_reduce(
                        rc[:, cl], psum[c2][0], channels=128,
                        reduce_op=bass_isa.ReduceOp.add)
                    nc.vector.reciprocal_approx_fast(rc[:, cl], rc[:, cl])
                # attention * V, transposed: aug[dh, q] += V_j^T P_j
                # (diagonal tiles stream only their causally-valid columns)
                for c2 in range(2):
                    jn = SUBS * w + 4 * (c2 + 1)
                    chunk0 = c2 * 512
                    aug = psag.tile([128, 512], fp32, tag="aug", name="aug")
                    for j in range(jn):
                        jj = j - SUBS * w - 4 * c2  # >=0: diagonal in chunk
                        a = max(0, jj) * 128
                        nc.tensor.matmul(
                            aug[:, a:512], v_sb[:, j, hs2],
                            pts[j][:, chunk0 + a:chunk0 + 512],
                            start=(j == 0), stop=(j == jn - 1))
                    nc.vector.tensor_mul(
                        oc[h][w][:, chunk0:chunk0 + 512], aug,
                        rc[:, chunk0:chunk0 + 512])
                    if after_chunk is not None:
                        after_chunk(w, c2)

            # ------- A2 (head-1 Q/K, half windows) ∥ B head-0 -------
            stX2 = ExitStack()
            xw2p = stX2.enter_context(tc.tile_pool(name="xw2", bufs=2))
            for w in range(N_QW):
                attn_window(0, w)
                for q in range(2 * w, 2 * (w + 1)):
                    sl = slice(q * PW, (q + 1) * PW)
                    xw = xw2p.tile([128, 16, PW], bf16, tag="xw2",
                                   name="xw2")
                    nc.sync.dma_start(out=xw[:, 0:8, :],
                                      in_=xT_r[:, q, 0:8, :])
                    nc.gpsimd.dma_start(out=xw[:, 8:16, :],
                                        in_=xT_r[:, q, 8:16, :])
                    project_qk(1, xw, sl, PW)
            stX2.close()

            # ---------------- B head-1 ∥ C ----------------
            stC = ExitStack()
            cst = stC.enter_context(tc.tile_pool(name="cst", bufs=3))

            def c_mtile(m):
                msl = slice((m % SUBS) * 128, (m % SUBS + 1) * 128)
                last = m >= S // 128 - SUBS
                so = cst.tile([128, D], bf16, tag="so", name="so")
                for nw in range(D // 512):
                    nsl = slice(nw * 512, (nw + 1) * 512)
                    ps = psA.tile([128, 512], fp32, tag="qk", name="cps")
                    for t in range(HPC):
                        nc.tensor.matmul(ps, oc[t][m // SUBS][:, msl],
                                         wo_sb[t][:, nsl],
                                         start=(t == 0), stop=(t == HPC - 1))
                    if nw % 2 == 0 and not last:
                        nc.vector.tensor_copy(so[:, nsl], ps)
                    else:
                        nc.scalar.copy(out=so[:, nsl], in_=ps)
                nc.sync.dma_start(out=out_d[m * 128:(m + 1) * 128, 0:1024],
                                  in_=so[:, 0:1024])
                nc.gpsimd.dma_start(
                    out=out_d[m * 128:(m + 1) * 128, 1024:2048],
                    in_=so[:, 1024:2048])

            def after_chunk(w, c2):
                for m in range(SUBS * w + 4 * c2, SUBS * w + 4 * (c2 + 1)):
                    c_mtile(m)

            for w in range(N_QW):
                attn_window(1, w, after_chunk=after_chunk)
            stC.close()
            stB.close()
            stA.close()

    with tile.TileContext(nc) as tc:
        for _ in range(reps):
            emit_body(tc)

    nc.compile()
    return nc


def _host_prep(inputs):
    x = np.ascontiguousarray(np.asarray(inputs["x"], dtype=np.float32)[0])  # [S, D]
    tp = np.asarray(inputs["token_positions"]).reshape(-1)[:S]
    Wq = np.asarray(inputs["Wq"], dtype=np.float32)
    Wk = np.asarray(inputs["Wk"], dtype=np.float32)
    Wv = np.asarray(inputs["Wv"], dtype=np.float32)
    Wo = np.asarray(inputs["Wo"], dtype=np.float32)

    # x^T in per-partition-contiguous window-major layout:
    # [p, w, t, s'] = x^T[t*128+p, w*512+s']  ->  [128, S*16]
    xT = np.ascontiguousarray(x.T).astype(BF16)  # [D, S]
    xTw = np.ascontiguousarray(
        xT.reshape(16, 128, S // 512, 512).transpose(1, 2, 0, 3)
    ).reshape(128, S * 16)

    # f32 RoPE tables, replicated across the two 64-row halves
    inv_freq = (10000.0 ** (-np.arange(0, DH, 2, dtype=np.float32) / DH)
                ).astype(np.float32)
    ang = tp.astype(np.float32)[:, None] * inv_freq[None, :]  # [S, 64] f32
    cos = np.cos(ang).astype(np.float32).T  # [64, S]
    sin = np.sin(ang).astype(np.float32).T
    cosF = np.concatenate([cos, cos], axis=0).astype(BF16)  # [128, S]
    sinX = np.concatenate([-sin, sin], axis=0).astype(BF16)
    # half-swap permutation as a matmul lhsT: out[m] = in[(m+64) % 128]
    swp = np.zeros((128, 128), dtype=np.float32)
    swp[np.arange(128), (np.arange(128) + 64) % 128] = 1.0
    swp = swp.astype(BF16)

    # causal mask in scores^T layout: valid iff k <= q  ->  upper triangular
    mask = np.triu(np.ones((128, 128), dtype=np.float32)).astype(BF16)

    perm = np.concatenate([np.arange(0, DH, 2), np.arange(1, DH, 2)])
    in_maps = []
    for c in range(N_CORES):
        rows = slice(c * HPC * DH, (c + 1) * HPC * DH)
        wq_blk = Wq[rows].reshape(HPC, DH, D)[:, perm].reshape(HPC * DH, D)
        wk_blk = Wk[rows].reshape(HPC, DH, D)[:, perm].reshape(HPC * DH, D)
        wv_blk = Wv[rows]
        def _wlay(blk):  # [256, D] -> [p, t, m] contiguous [128, 4096]
            bt = np.ascontiguousarray(blk.T).astype(BF16)  # [D, 256]
            return np.ascontiguousarray(
                bt.reshape(16, 128, HPC * DH).transpose(1, 0, 2)
            ).reshape(128, 16 * HPC * DH)

        in_maps.append({
            "xTw": xTw,
            "wqT": _wlay(wq_blk),
            "wkT": _wlay(wk_blk),
            "wvT": _wlay(wv_blk),
            "woT": np.ascontiguousarray(Wo[:, rows].T).astype(BF16),
            "cosF": cosF,
            "sinX": sinX,
            "swp": swp,
            "mask": mask,
        })
    return in_maps


def get_compiled():
    if "nc" not in _CACHE:
        _CACHE["nc"] = _build()
    return _CACHE["nc"]


def _get_runner(nc):
    """Build (once) a jitted 8-core runner; reused across kernel() calls."""
    if "runner" in _CACHE:
        return _CACHE["runner"]
    import jax
    from jax.sharding import Mesh, PartitionSpec
    from jax.experimental.shard_map import shard_map
    from concourse import bass2jax

    bass2jax.install_neuronx_cc_hook()
    part_name = (nc.partition_id_tensor.name
                 if nc.partition_id_tensor else None)
    in_names, out_names, out_avals, zero_outs = [], [], [], []
    for alloc in nc.m.functions[0].allocations:
        if not isinstance(alloc, mybir.MemoryLocationSet):
            continue
        name = alloc.memorylocations[0].name
        if alloc.kind == "ExternalInput":
            if name != part_name:
                in_names.append(name)
        elif alloc.kind == "ExternalOutput":
            shape = tuple(alloc.tensor_shape)
            dtype = mybir.dt.np(alloc.dtype)
            out_names.append(name)
            out_avals.append(jax.core.ShapedArray(shape, dtype))
            zero_outs.append(np.zeros(shape, dtype))
    n_params = len(in_names)
    all_in_names = list(in_names) + list(out_names)
    if part_name is not None:
        all_in_names = all_in_names + [part_name]

    def _body(*args):
        ins = list(args[:n_params])
        outs = list(args[n_params:])
        operands = ins + outs
        if part_name is not None:
            operands.append(bass2jax.partition_id_tensor())
        outs = list(bass2jax._bass_exec_p.bind(
            *operands,
            out_avals=tuple(out_avals),
            in_names=tuple(all_in_names),
            out_names=tuple(out_names),
            lowering_input_output_aliases=(),
            sim_require_finite=True,
            sim_require_nnan=True,
            nc=nc,
        ))
        return tuple(outs)

    devices = jax.devices()[:N_CORES]
    mesh = Mesh(np.asarray(devices), ("core",))
    nin = n_params + len(out_names)
    sharded = jax.jit(
        shard_map(_body, mesh=mesh,
                  in_specs=(PartitionSpec("core"),) * nin,
                  out_specs=(PartitionSpec("core"),) * len(out_names),
                  check_rep=False),
        keep_unused=True,
    )
    concat_zero = [np.zeros((N_CORES * z.shape[0], *z.shape[1:]), z.dtype)
                   for z in zero_outs]
    _CACHE["runner"] = (sharded, in_names, out_names, concat_zero)
    return _CACHE["runner"]


def kernel(**inputs):
    import jax
    nc = get_compiled()
    in_maps = _host_prep(inputs)
    sharded, in_names, out_names, concat_zero = _get_runner(nc)
    concat_in = [np.concatenate([np.asarray(in_maps[c][nm])
                                 for c in range(N_CORES)], axis=0)
                 for nm in in_names]
    out = sharded(*[jax.device_put(a) for a in concat_in + concat_zero])
    oi = out_names.index("out")
    res = np.asarray(out[oi]).reshape(N_CORES, S, D)
    y = res.astype(np.float32).sum(axis=0)
    return y.reshape(1, S, D)


# revision 31
# speedup vs baseline: 1.1207x; 1.1207x over previous
"""Multi-head self-attention (RoPE, causal) Trainium2 Bass kernel.

Sharding: tensor-parallel over heads. 16 heads / 8 cores = 2 heads per core.
Each core computes Q/K/V projections for its 2 heads, causal attention, and a
partial output projection against its 256-column slice of Wo. The host sums
the 8 partial [S, D] outputs.

All matmuls run in bf16 with fp32 PSUM accumulation. Softmax skips the max
subtraction (scores are O(1) for this problem family; exp stays in fp32
range). The attention*V phase is computed transposed — V tiles are the
stationary operand and P^T streams 512-wide q-chunks — so the PE streams full
free dims instead of 129-wide LDW-bound matmuls, and the output lands
directly in the [dh, q] layout the output projection wants (no transposes).
The softmax denominator comes from an f32 running sum of P^T tiles on the
vector engine, reduced across partitions by gpsimd.partition_all_reduce.
RoPE's even/odd interleave is folded into a row permutation of Wq/Wk, making
the on-device rotation a contiguous rotate-half.
"""
import sys

sys.path.insert(0, "/opt/trn_rl_repo")

import numpy as np
import ml_dtypes

import concourse.bass as bass  # noqa: F401  (registers AP machinery)
import concourse.tile as tile
from concourse import bacc, bass_isa, mybir
from concourse import bass_utils

BF16 = ml_dtypes.bfloat16
S = 4096
D = 2048
DH = 128
N_CORES = 8
HPC = 2  # heads per core
PW = 512  # projection s-window
QW = 1024  # attention q-window
N_QW = S // QW  # 4
SUBS = QW // 128  # 8 q-subtiles per window
N_KT = S // 128  # 32 k-tiles
INV_SQRT_DH = float(1.0 / np.sqrt(128.0))

_CACHE = {}


def _build(reps=1):
    fp32 = mybir.dt.float32
    bf16 = mybir.dt.bfloat16

    nc = bacc.Bacc("TRN2", target_bir_lowering=False, debug=False,
                   num_devices=N_CORES)
    xT_d = nc.dram_tensor("xTw", [128, S * 16], bf16,
                          kind="ExternalInput").ap()
    wq_d = nc.dram_tensor("wqT", [128, 16 * HPC * DH], bf16,
                          kind="ExternalInput").ap()
    wk_d = nc.dram_tensor("wkT", [128, 16 * HPC * DH], bf16,
                          kind="ExternalInput").ap()
    wv_d = nc.dram_tensor("wvT", [128, 16 * HPC * DH], bf16,
                          kind="ExternalInput").ap()
    wo_d = nc.dram_tensor("woT", [HPC * DH, D], bf16, kind="ExternalInput").ap()
    cos_d = nc.dram_tensor("cosF", [128, S], bf16, kind="ExternalInput").ap()
    sin_d = nc.dram_tensor("sinX", [128, S], bf16, kind="ExternalInput").ap()
    swp_d = nc.dram_tensor("swp", [128, 128], bf16, kind="ExternalInput").ap()
    mask_d = nc.dram_tensor("mask", [128, 128], bf16, kind="ExternalInput").ap()
    out_d = nc.dram_tensor("out", [S, D], bf16, kind="ExternalOutput").ap()

    xT_r = xT_d.rearrange("p (w t s) -> p w t s", w=S // PW,
                          t=16)               # [128, 8, 16, 512]
    wq_r = wq_d.rearrange("p (t m) -> p t m", t=16)    # [128, 16, 256]
    wk_r = wk_d.rearrange("p (t m) -> p t m", t=16)
    wv_r = wv_d.rearrange("p (t m) -> p t m", t=16)

    EXP = mybir.ActivationFunctionType.Exp

    from contextlib import ExitStack

    def emit_body(tc):
        with tc.tile_pool(name="persist", bufs=1) as pp, \
             tc.tile_pool(name="ropet", bufs=1) as rtp:
            # ---- persistent tiles + initial loads (spread across engines)
            qt = [pp.tile([128, S], bf16, tag=f"qt{h}", name=f"qt{h}")
                  for h in range(HPC)]
            kt = [pp.tile([128, S], bf16, tag=f"kt{h}", name=f"kt{h}")
                  for h in range(HPC)]
            v_sb = pp.tile([128, N_KT, HPC * DH], bf16, tag="v")
            oc = [[pp.tile([128, QW], bf16, tag=f"oc{h}w{w}", name=f"oc{h}w{w}")
                   for w in range(N_QW)] for h in range(HPC)]
            cos_sb = pp.tile([128, S], bf16, tag="cos")
            sin_sb = pp.tile([128, S], bf16, tag="sin")
            mask_sb = pp.tile([128, 128], bf16, tag="mask")
            swp_sb = pp.tile([128, 128], bf16, tag="swp")

            stA = ExitStack()
            psA = stA.enter_context(
                tc.tile_pool(name="psA", bufs=2, space="PSUM"))
            wqkp = stA.enter_context(tc.tile_pool(name="wqk", bufs=1))
            wq_sb = wqkp.tile([128, 16, HPC * DH], bf16, tag="wq")
            wk_sb = wqkp.tile([128, 16, HPC * DH], bf16, tag="wk")
            stV = ExitStack()
            psV = stV.enter_context(
                tc.tile_pool(name="psV", bufs=2, space="PSUM"))
            wvp = stV.enter_context(tc.tile_pool(name="wvp", bufs=1))
            xw1p = stV.enter_context(tc.tile_pool(name="xw1", bufs=3))
            wv_sb = wvp.tile([128, 16, HPC * DH], bf16, tag="wv")

            nc.sync.dma_start(out=wq_sb, in_=wq_r)
            nc.gpsimd.dma_start(out=wk_sb, in_=wk_r)
            nc.scalar.dma_start(out=wv_sb, in_=wv_r)
            nc.scalar.dma_start(out=swp_sb, in_=swp_d)
            nc.scalar.dma_start(out=cos_sb, in_=cos_d)
            nc.scalar.dma_start(out=sin_sb, in_=sin_d)

            wo_sb = [pp.tile([128, D], bf16, tag=f"wo{t}", name=f"wo{t}")
                     for t in range(HPC)]

            def project_qk(h, xw, sl, pw):
                hs = slice(h * DH, (h + 1) * DH)
                for wsb, dest in ((wq_sb, qt[h]), (wk_sb, kt[h])):
                    ps = psA.tile([128, pw], fp32, tag="qk", name="ps",
                                  padded_shape=[128, PW])
                    for t in range(16):
                        nc.tensor.matmul(ps, wsb[:, t, hs], xw[:, t, :],
                                         start=(t == 0), stop=(t == 15))
                    if h == 0:
                        nc.scalar.copy(out=dest[:, sl], in_=ps)
                    else:
                        nc.vector.tensor_copy(dest[:, sl], ps)
                    # rope in place: dest = dest*cosF + swap(dest)*[-sin;sin]
                    dsl = dest[:, sl]
                    swp = psA.tile([128, pw], fp32, tag="qk", bufs=2,
                                   name="swp", padded_shape=[128, PW])
                    nc.tensor.matmul(swp, swp_sb, dsl, start=True, stop=True)
                    m1 = rtp.tile([128, pw], bf16, tag="m1", name="m1",
                                  padded_shape=[128, PW])
                    m2 = rtp.tile([128, pw], bf16, tag="m2", name="m2",
                                  padded_shape=[128, PW])
                    nc.vector.tensor_mul(m1, dsl, cos_sb[:, sl])
                    nc.vector.tensor_mul(m2, swp, sin_sb[:, sl])
                    nc.vector.tensor_add(dsl, m1, m2)

            # ---------------- A1: head-0 Q/K + all V ----------------
            # first 512-window split into 256 halves so the very first
            # projection starts after ~1.5 MB of DMA; rope tables stream
            # in per-piece slices; mask/wo defer a few windows in
            pieces = [(0, 256), (256, 256)] + [
                (k * PW, PW) for k in range(1, S // PW)]
            for pi, (p0, pw) in enumerate(pieces):
                sl = slice(p0, p0 + pw)
                wi, wo_off = p0 // PW, p0 % PW
                xw = xw1p.tile([128, 16, pw], bf16, tag="xw",
                               name="xw", padded_shape=[128, 16, PW])
                nc.sync.dma_start(
                    out=xw[:, 0:8, :],
                    in_=xT_r[:, wi, 0:8, wo_off:wo_off + pw])
                nc.gpsimd.dma_start(
                    out=xw[:, 8:16, :],
                    in_=xT_r[:, wi, 8:16, wo_off:wo_off + pw])
                if pi == 4:
                    nc.scalar.dma_start(out=mask_sb, in_=mask_d)
                elif pi == 5:
                    for t in range(HPC):
                        nc.scalar.dma_start(
                            out=wo_sb[t], in_=wo_d[t * 128:(t + 1) * 128, :])
                project_qk(0, xw, sl, pw)
                for sub in range(pw // 128):
                    st = p0 // 128 + sub
                    ssl = slice(sub * 128, (sub + 1) * 128)
                    pv = psV.tile([128, HPC * DH], fp32, tag="v")
                    for t in range(16):
                        nc.tensor.matmul(pv, xw[:, t, ssl], wv_sb[:, t, :],
                                         start=(t == 0), stop=(t == 15))
                    nc.scalar.copy(out=v_sb[:, st, :], in_=pv)
            stV.close()

            # ---------------- B machinery ----------------
            stB = ExitStack()
            ptp = stB.enter_context(tc.tile_pool(name="pt", bufs=1))
            psmp = stB.enter_context(tc.tile_pool(name="psm", bufs=1))
            rcpp = stB.enter_context(tc.tile_pool(name="rcp", bufs=2))
            pssc = stB.enter_context(
                tc.tile_pool(name="pssc", bufs=2, space="PSUM"))
            psag = stB.enter_context(
                tc.tile_pool(name="psag", bufs=2, space="PSUM"))

            def attn_window(h, w, after_chunk=None):
                hs2 = slice(h * DH, (h + 1) * DH)
                q0 = w * QW
                n_j = SUBS * w + SUBS
                par = [2 if w > 0 else 1, 2]  # accumulators in use per chunk
                psum = [[psmp.tile([128, 512], bf16, tag=f"psum{c2}p{p}",
                                   name=f"psum{c2}p{p}")
                         for p in range(par[c2])] for c2 in range(2)]
                nacc = [0, 0]  # contributions so far per chunk
                pts = []
                for j in range(n_j):
                    ksl = slice(j * 128, (j + 1) * 128)
                    c = j - SUBS * w  # >= 0 -> diagonal strip
                    lo = max(0, c) * 128
                    sc = pssc.tile([128, QW], fp32, tag="sc", name="sc")
                    if lo < 512:
                        nc.tensor.matmul(sc[:, lo:512], kt[h][:, ksl],
                                         qt[h][:, q0 + lo:q0 + 512],
                                         start=True, stop=True)
                        nc.tensor.matmul(sc[:, 512:1024], kt[h][:, ksl],
                                         qt[h][:, q0 + 512:q0 + 1024],
                                         start=True, stop=True)
                    else:
                        nc.tensor.matmul(sc[:, lo:1024], kt[h][:, ksl],
                                         qt[h][:, q0 + lo:q0 + 1024],
                                         start=True, stop=True)
                    pt = ptp.tile([128, QW], bf16, tag=f"pt{j}",
                                  name=f"pt{j}")
                    nc.scalar.activation(pt[:, lo:], sc[:, lo:], EXP,
                                         scale=INV_SQRT_DH)
                    if c >= 0:
                        csl = slice(c * 128, (c + 1) * 128)
                        nc.vector.tensor_mul(pt[:, csl], pt[:, csl], mask_sb)
                    # running bf16 denominator sums: per 512-chunk (so
                    # chunk-0's D closes 4 exps early) and parity-split to
                    # halve the serial DVE chain; valid columns only
                    for c2 in range(2):
                        a = max(lo - c2 * 512, 0)
                        if a >= 512:
                            continue
                        acc = psum[c2][nacc[c2] % par[c2]]
                        srcp = pt[:, c2 * 512 + a:(c2 + 1) * 512]
                        if nacc[c2] < par[c2]:
                            assert a == 0 or par[c2] == 1 or nacc[c2] == 0
                            if a == 0:
                                nc.vector.tensor_copy(
                                    acc, pt[:, c2 * 512:(c2 + 1) * 512])
                            else:
                                nc.vector.tensor_copy(acc[:, a:], srcp)
                                nc.vector.memset(acc[:, 0:a], 0.0)
                        else:
                            nc.vector.tensor_add(acc[:, a:], acc[:, a:],
                                                 srcp)
                        nacc[c2] += 1
                    pts.append(pt)
                # denominator: all-partitions sum (f32 internally) -> recip
                rc = rcpp.tile([128, QW], fp32, tag="rc", name="rc")
                for c2 in range(2):
                    cl = slice(c2 * 512, (c2 + 1) * 512)
                    if par[c2] == 2:
                        nc.vector.tensor_add(psum[c2][0], psum[c2][0],
                                             psum[c2][1])
                    nc.gpsimd.partition_all_reduce(
                        rc[:, cl], psum[c2][0], channels=128,
                        reduce_op=bass_isa.ReduceOp.add)
                    nc.vector.reciprocal_approx_fast(rc[:, cl], rc[:, cl])
                # attention * V, transposed: aug[dh, q] += V_j^T P_j
                # (diagonal tiles stream only their causally-valid columns)
                for c2 in range(2):
                    jn = SUBS * w + 4 * (c2 + 1)
                    chunk0 = c2 * 512
                    aug = psag.tile([128, 512], fp32, tag="aug", name="aug")
                    for j in range(jn):
                        jj = j - SUBS * w - 4 * c2  # >=0: diagonal in chunk
                        a = max(0, jj) * 128
                        nc.tensor.matmul(
                            aug[:, a:512], v_sb[:, j, hs2],
                            pts[j][:, chunk0 + a:chunk0 + 512],
                            start=(j == 0), stop=(j == jn - 1))
                    nc.vector.tensor_mul(
                        oc[h][w][:, chunk0:chunk0 + 512], aug,
                        rc[:, chunk0:chunk0 + 512])
                    if after_chunk is not None:
                        after_chunk(w, c2)

            # ------- A2 (head-1 Q/K, quarter windows) ∥ B head-0 -------
            PW2 = 256
            stX2 = ExitStack()
            xw2p = stX2.enter_context(tc.tile_pool(name="xw2", bufs=2))
            n_q = S // PW2  # 16 quarter windows
            for w in range(N_QW):
                attn_window(0, w)
                for q in range(n_q // N_QW * w, n_q // N_QW * (w + 1)):
                    sl = slice(q * PW2, (q + 1) * PW2)
                    ho = (q % 2) * PW2
                    xw = xw2p.tile([128, 16, PW2], bf16, tag="xw2",
                                   name="xw2")
                    nc.sync.dma_start(out=xw[:, 0:8, :],
                                      in_=xT_r[:, q // 2, 0:8, ho:ho + PW2])
                    nc.gpsimd.dma_start(
                        out=xw[:, 8:16, :],
                        in_=xT_r[:, q // 2, 8:16, ho:ho + PW2])
                    project_qk(1, xw, sl, PW2)
            stX2.close()

            # ---------------- B head-1 ∥ C ----------------
            stC = ExitStack()
            cst = stC.enter_context(tc.tile_pool(name="cst", bufs=3))

            def c_mtile(m):
                msl = slice((m % SUBS) * 128, (m % SUBS + 1) * 128)
                last = m >= S // 128 - SUBS
                so = cst.tile([128, D], bf16, tag="so", name="so")
                for nw in range(D // 512):
                    nsl = slice(nw * 512, (nw + 1) * 512)
                    ps = psA.tile([128, 512], fp32, tag="qk", name="cps")
                    for t in range(HPC):
                        nc.tensor.matmul(ps, oc[t][m // SUBS][:, msl],
                                         wo_sb[t][:, nsl],
                                         start=(t == 0), stop=(t == HPC - 1))
                    if nw % 2 == 0 and not last:
                        nc.vector.tensor_copy(so[:, nsl], ps)
                    else:
                        nc.scalar.copy(out=so[:, nsl], in_=ps)
                nc.sync.dma_start(out=out_d[m * 128:(m + 1) * 128, 0:1024],
                                  in_=so[:, 0:1024])
                nc.gpsimd.dma_start(
                    out=out_d[m * 128:(m + 1) * 128, 1024:2048],
                    in_=so[:, 1024:2048])

            def after_chunk(w, c2):
                for m in range(SUBS * w + 4 * c2, SUBS * w + 4 * (c2 + 1)):
                    c_mtile(m)

            for w in range(N_QW):
                attn_window(1, w, after_chunk=after_chunk)
            stC.close()
            stB.close()
            stA.close()

    with tile.TileContext(nc) as tc:
        for _ in range(reps):
            emit_body(tc)

    nc.compile()
    return nc


def _host_prep(inputs):
    x = np.ascontiguousarray(np.asarray(inputs["x"], dtype=np.float32)[0])  # [S, D]
    tp = np.asarray(inputs["token_positions"]).reshape(-1)[:S]
    Wq = np.asarray(inputs["Wq"], dtype=np.float32)
    Wk = np.asarray(inputs["Wk"], dtype=np.float32)
    Wv = np.asarray(inputs["Wv"], dtype=np.float32)
    Wo = np.asarray(inputs["Wo"], dtype=np.float32)

    # x^T in per-partition-contiguous window-major layout:
    # [p, w, t, s'] = x^T[t*128+p, w*512+s']  ->  [128, S*16]
    xT = np.ascontiguousarray(x.T).astype(BF16)  # [D, S]
    xTw = np.ascontiguousarray(
        xT.reshape(16, 128, S // 512, 512).transpose(1, 2, 0, 3)
    ).reshape(128, S * 16)

    # f32 RoPE tables, replicated across the two 64-row halves
    inv_freq = (10000.0 ** (-np.arange(0, DH, 2, dtype=np.float32) / DH)
                ).astype(np.float32)
    ang = tp.astype(np.float32)[:, None] * inv_freq[None, :]  # [S, 64] f32
    cos = np.cos(ang).astype(np.float32).T  # [64, S]
    sin = np.sin(ang).astype(np.float32).T
    cosF = np.concatenate([cos, cos], axis=0).astype(BF16)  # [128, S]
    sinX = np.concatenate([-sin, sin], axis=0).astype(BF16)
    # half-swap permutation as a matmul lhsT: out[m] = in[(m+64) % 128]
    swp = np.zeros((128, 128), dtype=np.float32)
    swp[np.arange(128), (np.arange(128) + 64) % 128] = 1.0
    swp = swp.astype(BF16)

    # causal mask in scores^T layout: valid iff k <= q  ->  upper triangular
    mask = np.triu(np.ones((128, 128), dtype=np.float32)).astype(BF16)

    perm = np.concatenate([np.arange(0, DH, 2), np.arange(1, DH, 2)])
    in_maps = []
    for c in range(N_CORES):
        rows = slice(c * HPC * DH, (c + 1) * HPC * DH)
        wq_blk = Wq[rows].reshape(HPC, DH, D)[:, perm].reshape(HPC * DH, D)
        wk_blk = Wk[rows].reshape(HPC, DH, D)[:, perm].reshape(HPC * DH, D)
        wv_blk = Wv[rows]
        def _wlay(blk):  # [256, D] -> [p, t, m] contiguous [128, 4096]
            bt = np.ascontiguousarray(blk.T).astype(BF16)  # [D, 256]
            return np.ascontiguousarray(
                bt.reshape(16, 128, HPC * DH).transpose(1, 0, 2)
            ).reshape(128, 16 * HPC * DH)

        in_maps.append({
            "xTw": xTw,
            "wqT": _wlay(wq_blk),
            "wkT": _wlay(wk_blk),
            "wvT": _wlay(wv_blk),
            "woT": np.ascontiguousarray(Wo[:, rows].T).astype(BF16),
            "cosF": cosF,
            "sinX": sinX,
            "swp": swp,
            "mask": mask,
        })
    return in_maps


def get_compiled():
    if "nc" not in _CACHE:
        _CACHE["nc"] = _build()
    return _CACHE["nc"]


def _get_runner(nc):
    """Build (once) a jitted 8-core runner; reused across kernel() calls."""
    if "runner" in _CACHE:
        return _CACHE["runner"]
    import jax
    from jax.sharding import Mesh, PartitionSpec
    from jax.experimental.shard_map import shard_map
    from concourse import bass2jax

    bass2jax.install_neuronx_cc_hook()
    part_name = (nc.partition_id_tensor.name
                 if nc.partition_id_tensor else None)
    in_names, out_names, out_avals, zero_outs = [], [], [], []
    for alloc in nc.m.functions[0].allocations:
        if not isinstance(alloc, mybir.MemoryLocationSet):
            continue
        name = alloc.memorylocations[0].name
        if alloc.kind == "ExternalInput":
            if name != part_name:
                in_names.append(name)
        elif alloc.kind == "ExternalOutput":
            shape = tuple(alloc.tensor_shape)
            dtype = mybir.dt.np(alloc.dtype)
            out_names.append(name)
            out_avals.append(jax.core.ShapedArray(shape, dtype))
            zero_outs.append(np.zeros(shape, dtype))
    n_params = len(in_names)
    all_in_names = list(in_names) + list(out_names)
    if part_name is not None:
        all_in_names = all_in_names + [part_name]

    def _body(*args):
        ins = list(args[:n_params])
        outs = list(args[n_params:])
        operands = ins + outs
        if part_name is not None:
            operands.append(bass2jax.partition_id_tensor())
        outs = list(bass2jax._bass_exec_p.bind(
            *operands,
            out_avals=tuple(out_avals),
            in_names=tuple(all_in_names),
            out_names=tuple(out_names),
            lowering_input_output_aliases=(),
            sim_require_finite=True,
            sim_require_nnan=True,
            nc=nc,
        ))
        return tuple(outs)

    devices = jax.devices()[:N_CORES]
    mesh = Mesh(np.asarray(devices), ("core",))
    nin = n_params + len(out_names)
    sharded = jax.jit(
        shard_map(_body, mesh=mesh,
                  in_specs=(PartitionSpec("core"),) * nin,
                  out_specs=(PartitionSpec("core"),) * len(out_names),
                  check_rep=False),
        keep_unused=True,
    )
    concat_zero = [np.zeros((N_CORES * z.shape[0], *z.shape[1:]), z.dtype)
                   for z in zero_outs]
    _CACHE["runner"] = (sharded, in_names, out_names, concat_zero)
    return _CACHE["runner"]


def kernel(**inputs):
    import jax
    nc = get_compiled()
    in_maps = _host_prep(inputs)
    sharded, in_names, out_names, concat_zero = _get_runner(nc)
    concat_in = [np.concatenate([np.asarray(in_maps[c][nm])
                                 for c in range(N_CORES)], axis=0)
                 for nm in in_names]
    out = sharded(*[jax.device_put(a) for a in concat_in + concat_zero])
    oi = out_names.index("out")
    res = np.asarray(out[oi]).reshape(N_CORES, S, D)
    y = res.astype(np.float32).sum(axis=0)
    return y.reshape(1, S, D)


# revision 34
# speedup vs baseline: 2.2274x; 1.9875x over previous
"""Multi-head self-attention (RoPE, causal) Trainium2 Bass kernel.

Sharding: tensor-parallel over heads. 16 heads / 8 cores = 2 heads per core.
Each core computes Q/K/V projections for its 2 heads, causal attention, and a
partial output projection against its 256-column slice of Wo. The host sums
the 8 partial [S, D] outputs.

All matmuls run in bf16 with fp32 PSUM accumulation. Softmax skips the max
subtraction (scores are O(1) for this problem family; exp stays in fp32
range). The attention*V phase is computed transposed — V tiles are the
stationary operand and P^T streams 512-wide q-chunks — so the PE streams full
free dims instead of 129-wide LDW-bound matmuls, and the output lands
directly in the [dh, q] layout the output projection wants (no transposes).
The softmax denominator comes from an f32 running sum of P^T tiles on the
vector engine, reduced across partitions by gpsimd.partition_all_reduce.
RoPE's even/odd interleave is folded into a row permutation of Wq/Wk, making
the on-device rotation a contiguous rotate-half.
"""
import sys

sys.path.insert(0, "/opt/trn_rl_repo")

import numpy as np
import ml_dtypes

import concourse.bass as bass  # noqa: F401  (registers AP machinery)
import concourse.tile as tile
from concourse import bacc, bass_isa, mybir
from concourse import bass_utils

BF16 = ml_dtypes.bfloat16
S = 4096
D = 2048
DH = 128
N_CORES = 8
HPC = 2  # heads per core
PW = 512  # projection s-window
QW = 1024  # attention q-window
N_QW = S // QW  # 4
SUBS = QW // 128  # 8 q-subtiles per window
N_KT = S // 128  # 32 k-tiles
INV_SQRT_DH = float(1.0 / np.sqrt(128.0))

_CACHE = {}


def _build(reps=1):
    fp32 = mybir.dt.float32
    bf16 = mybir.dt.bfloat16

    nc = bacc.Bacc("TRN2", target_bir_lowering=False, debug=False,
                   num_devices=N_CORES)
    xT_d = nc.dram_tensor("xTw", [128, S * 16], bf16,
                          kind="ExternalInput").ap()
    wq_d = nc.dram_tensor("wqT", [128, 16 * HPC * DH], bf16,
                          kind="ExternalInput").ap()
    wk_d = nc.dram_tensor("wkT", [128, 16 * HPC * DH], bf16,
                          kind="ExternalInput").ap()
    wv_d = nc.dram_tensor("wvT", [128, 16 * HPC * DH], bf16,
                          kind="ExternalInput").ap()
    wo_d = nc.dram_tensor("woT", [HPC * DH, D], bf16, kind="ExternalInput").ap()
    cos_d = nc.dram_tensor("cosF", [128, S], bf16, kind="ExternalInput").ap()
    sin_d = nc.dram_tensor("sinX", [128, S], bf16, kind="ExternalInput").ap()
    swp_d = nc.dram_tensor("swp", [128, 128], bf16, kind="ExternalInput").ap()
    mask_d = nc.dram_tensor("mask", [128, 128], bf16, kind="ExternalInput").ap()
    out_d = nc.dram_tensor("out", [S, D], bf16, kind="ExternalOutput").ap()

    xT_r = xT_d.rearrange("p (w t s) -> p w t s", w=S // PW,
                          t=16)               # [128, 8, 16, 512]
    wq_r = wq_d.rearrange("p (t m) -> p t m", t=16)    # [128, 16, 256]
    wk_r = wk_d.rearrange("p (t m) -> p t m", t=16)
    wv_r = wv_d.rearrange("p (t m) -> p t m", t=16)

    EXP = mybir.ActivationFunctionType.Exp

    from contextlib import ExitStack

    def emit_body(tc):
        with tc.tile_pool(name="persist", bufs=1) as pp, \
             tc.tile_pool(name="ropet", bufs=1) as rtp:
            # ---- persistent tiles + initial loads (spread across engines)
            qt = [pp.tile([128, S], bf16, tag=f"qt{h}", name=f"qt{h}")
                  for h in range(HPC)]
            kt = [pp.tile([128, S], bf16, tag=f"kt{h}", name=f"kt{h}")
                  for h in range(HPC)]
            v_sb = pp.tile([128, N_KT, HPC * DH], bf16, tag="v")
            oc = [[pp.tile([128, QW], bf16, tag=f"oc{h}w{w}", name=f"oc{h}w{w}")
                   for w in range(N_QW)] for h in range(HPC)]
            cos_sb = pp.tile([128, S], bf16, tag="cos")
            sin_sb = pp.tile([128, S], bf16, tag="sin")
            mask_sb = pp.tile([128, 128], bf16, tag="mask")
            swp_sb = pp.tile([128, 128], bf16, tag="swp")

            stA = ExitStack()
            psA = stA.enter_context(
                tc.tile_pool(name="psA", bufs=2, space="PSUM"))
            wqkp = stA.enter_context(tc.tile_pool(name="wqk", bufs=1))
            wq_sb = wqkp.tile([128, 16, HPC * DH], bf16, tag="wq")
            wk_sb = wqkp.tile([128, 16, HPC * DH], bf16, tag="wk")
            stV = ExitStack()
            psV = stV.enter_context(
                tc.tile_pool(name="psV", bufs=2, space="PSUM"))
            wvp = stV.enter_context(tc.tile_pool(name="wvp", bufs=1))
            xw1p = stV.enter_context(tc.tile_pool(name="xw1", bufs=3))
            wv_sb = wvp.tile([128, 16, HPC * DH], bf16, tag="wv")

            nc.sync.dma_start(out=wq_sb, in_=wq_r)
            nc.gpsimd.dma_start(out=wk_sb, in_=wk_r)
            nc.scalar.dma_start(out=wv_sb, in_=wv_r)
            nc.scalar.dma_start(out=swp_sb, in_=swp_d)
            nc.scalar.dma_start(out=cos_sb, in_=cos_d)
            nc.scalar.dma_start(out=sin_sb, in_=sin_d)

            wo_sb = [pp.tile([128, D], bf16, tag=f"wo{t}", name=f"wo{t}")
                     for t in range(HPC)]

            def project_qk(h, xw, sl, pw):
                hs = slice(h * DH, (h + 1) * DH)
                for wsb, dest in ((wq_sb, qt[h]), (wk_sb, kt[h])):
                    ps = psA.tile([128, pw], fp32, tag="qk", name="ps",
                                  padded_shape=[128, PW])
                    for t in range(16):
                        nc.tensor.matmul(ps, wsb[:, t, hs], xw[:, t, :],
                                         start=(t == 0), stop=(t == 15))
                    if h == 0:
                        nc.scalar.copy(out=dest[:, sl], in_=ps)
                    else:
                        nc.vector.tensor_copy(dest[:, sl], ps)
                    # rope in place: dest = dest*cosF + swap(dest)*[-sin;sin]
                    dsl = dest[:, sl]
                    swp = psA.tile([128, pw], fp32, tag="qk", bufs=2,
                                   name="swp", padded_shape=[128, PW])
                    nc.tensor.matmul(swp, swp_sb, dsl, start=True, stop=True)
                    m1 = rtp.tile([128, pw], bf16, tag="m1", name="m1",
                                  padded_shape=[128, PW])
                    m2 = rtp.tile([128, pw], bf16, tag="m2", name="m2",
                                  padded_shape=[128, PW])
                    nc.vector.tensor_mul(m1, dsl, cos_sb[:, sl])
                    nc.vector.tensor_mul(m2, swp, sin_sb[:, sl])
                    nc.vector.tensor_add(dsl, m1, m2)

            # ---------------- A1: head-0 Q/K + all V ----------------
            # first 512-window split into 256 halves so the very first
            # projection starts after ~1.5 MB of DMA; rope tables stream
            # in per-piece slices; mask/wo defer a few windows in
            pieces = [(0, 256), (256, 256)] + [
                (k * PW, PW) for k in range(1, S // PW)]
            for pi, (p0, pw) in enumerate(pieces):
                sl = slice(p0, p0 + pw)
                wi, wo_off = p0 // PW, p0 % PW
                xw = xw1p.tile([128, 16, pw], bf16, tag="xw",
                               name="xw", padded_shape=[128, 16, PW])
                nc.sync.dma_start(
                    out=xw[:, 0:8, :],
                    in_=xT_r[:, wi, 0:8, wo_off:wo_off + pw])
                nc.gpsimd.dma_start(
                    out=xw[:, 8:16, :],
                    in_=xT_r[:, wi, 8:16, wo_off:wo_off + pw])
                if pi == 4:
                    nc.scalar.dma_start(out=mask_sb, in_=mask_d)
                elif pi == 5:
                    for t in range(HPC):
                        nc.scalar.dma_start(
                            out=wo_sb[t], in_=wo_d[t * 128:(t + 1) * 128, :])
                project_qk(0, xw, sl, pw)
                for sub in range(pw // 128):
                    st = p0 // 128 + sub
                    ssl = slice(sub * 128, (sub + 1) * 128)
                    pv = psV.tile([128, HPC * DH], fp32, tag="v")
                    for t in range(16):
                        nc.tensor.matmul(pv, xw[:, t, ssl], wv_sb[:, t, :],
                                         start=(t == 0), stop=(t == 15))
                    nc.scalar.copy(out=v_sb[:, st, :], in_=pv)
            stV.close()

            # ---------------- B machinery ----------------
            stB = ExitStack()
            ptp = stB.enter_context(tc.tile_pool(name="pt", bufs=1))
            psmp = stB.enter_context(tc.tile_pool(name="psm", bufs=2))
            rcpp = stB.enter_context(tc.tile_pool(name="rcp", bufs=3))
            pssc = stB.enter_context(
                tc.tile_pool(name="pssc", bufs=2, space="PSUM"))
            psag = stB.enter_context(
                tc.tile_pool(name="psag", bufs=2, space="PSUM"))

            def attn_scores(h, w):
                q0 = w * QW
                n_j = SUBS * w + SUBS
                par = [2 if w > 0 else 1, 2]  # accumulators in use per chunk
                psum = [[psmp.tile([128, 512], bf16, tag=f"psum{c2}p{p}",
                                   name=f"psum{c2}p{p}")
                         for p in range(par[c2])] for c2 in range(2)]
                nacc = [0, 0]  # contributions so far per chunk
                pts = []
                for j in range(n_j):
                    ksl = slice(j * 128, (j + 1) * 128)
                    c = j - SUBS * w  # >= 0 -> diagonal strip
                    lo = max(0, c) * 128
                    sc = pssc.tile([128, QW], fp32, tag="sc", name="sc")
                    if lo < 512:
                        nc.tensor.matmul(sc[:, lo:512], kt[h][:, ksl],
                                         qt[h][:, q0 + lo:q0 + 512],
                                         start=True, stop=True)
                        nc.tensor.matmul(sc[:, 512:1024], kt[h][:, ksl],
                                         qt[h][:, q0 + 512:q0 + 1024],
                                         start=True, stop=True)
                    else:
                        nc.tensor.matmul(sc[:, lo:1024], kt[h][:, ksl],
                                         qt[h][:, q0 + lo:q0 + 1024],
                                         start=True, stop=True)
                    pt = ptp.tile([128, QW], bf16, tag=f"pt{j}",
                                  name=f"pt{j}")
                    nc.scalar.activation(pt[:, lo:], sc[:, lo:], EXP,
                                         scale=INV_SQRT_DH)
                    if c >= 0:
                        csl = slice(c * 128, (c + 1) * 128)
                        nc.vector.tensor_mul(pt[:, csl], pt[:, csl], mask_sb)
                    # running bf16 denominator sums: per 512-chunk (so
                    # chunk-0's D closes 4 exps early) and parity-split to
                    # halve the serial DVE chain; valid columns only
                    for c2 in range(2):
                        a = max(lo - c2 * 512, 0)
                        if a >= 512:
                            continue
                        acc = psum[c2][nacc[c2] % par[c2]]
                        srcp = pt[:, c2 * 512 + a:(c2 + 1) * 512]
                        if nacc[c2] < par[c2]:
                            assert a == 0 or par[c2] == 1 or nacc[c2] == 0
                            if a == 0:
                                nc.vector.tensor_copy(
                                    acc, pt[:, c2 * 512:(c2 + 1) * 512])
                            else:
                                nc.vector.tensor_copy(acc[:, a:], srcp)
                                nc.vector.memset(acc[:, 0:a], 0.0)
                        else:
                            nc.vector.tensor_add(acc[:, a:], acc[:, a:],
                                                 srcp)
                        nacc[c2] += 1
                    pts.append(pt)
                # denominator: all-partitions sum (f32 internally) -> recip
                rc = rcpp.tile([128, QW], fp32, tag="rc", name="rc")
                for c2 in range(2):
                    cl = slice(c2 * 512, (c2 + 1) * 512)
                    if par[c2] == 2:
                        nc.vector.tensor_add(psum[c2][0], psum[c2][0],
                                             psum[c2][1])
                    nc.gpsimd.partition_all_reduce(
                        rc[:, cl], psum[c2][0], channels=128,
                        reduce_op=bass_isa.ReduceOp.add)
                    nc.vector.reciprocal_approx_fast(rc[:, cl], rc[:, cl])
                return pts, rc

            def attn_av(h, w, pts, rc, after_chunk=None):
                hs2 = slice(h * DH, (h + 1) * DH)
                # attention * V, transposed: aug[dh, q] += V_j^T P_j
                # (diagonal tiles stream only their causally-valid columns)
                for c2 in range(2):
                    jn = SUBS * w + 4 * (c2 + 1)
                    chunk0 = c2 * 512
                    aug = psag.tile([128, 512], fp32, tag="aug", name="aug")
                    for j in range(jn):
                        jj = j - SUBS * w - 4 * c2  # >=0: diagonal in chunk
                        a = max(0, jj) * 128
                        nc.tensor.matmul(
                            aug[:, a:512], v_sb[:, j, hs2],
                            pts[j][:, chunk0 + a:chunk0 + 512],
                            start=(j == 0), stop=(j == jn - 1))
                    nc.vector.tensor_mul(
                        oc[h][w][:, chunk0:chunk0 + 512], aug,
                        rc[:, chunk0:chunk0 + 512])
                    if after_chunk is not None:
                        after_chunk(w, c2)

            def attn_window(h, w, after_chunk=None):
                pts, rc = attn_scores(h, w)
                attn_av(h, w, pts, rc, after_chunk=after_chunk)

            # ------- A2 (head-1 Q/K, quarter windows) ∥ B head-0 -------
            PW2 = 256
            stX2 = ExitStack()
            xw2p = stX2.enter_context(tc.tile_pool(name="xw2", bufs=2))
            n_q = S // PW2  # 16 quarter windows
            for w in range(N_QW):
                attn_window(0, w)
                for q in range(n_q // N_QW * w, n_q // N_QW * (w + 1)):
                    sl = slice(q * PW2, (q + 1) * PW2)
                    ho = (q % 2) * PW2
                    xw = xw2p.tile([128, 16, PW2], bf16, tag="xw2",
                                   name="xw2")
                    nc.sync.dma_start(out=xw[:, 0:8, :],
                                      in_=xT_r[:, q // 2, 0:8, ho:ho + PW2])
                    nc.gpsimd.dma_start(
                        out=xw[:, 8:16, :],
                        in_=xT_r[:, q // 2, 8:16, ho:ho + PW2])
                    project_qk(1, xw, sl, PW2)
            stX2.close()

            # ---------------- B head-1 ∥ C ----------------
            stC = ExitStack()
            cst = stC.enter_context(tc.tile_pool(name="cst", bufs=4))

            def c_mtile(m):
                msl = slice((m % SUBS) * 128, (m % SUBS + 1) * 128)
                last = m >= S // 128 - SUBS
                so = cst.tile([128, D], bf16, tag="so", name="so")
                for nw in range(D // 512):
                    nsl = slice(nw * 512, (nw + 1) * 512)
                    ps = psA.tile([128, 512], fp32, tag="qk", name="cps")
                    for t in range(HPC):
                        nc.tensor.matmul(ps, oc[t][m // SUBS][:, msl],
                                         wo_sb[t][:, nsl],
                                         start=(t == 0), stop=(t == HPC - 1))
                    if nw % 2 == 0 and not last:
                        nc.vector.tensor_copy(so[:, nsl], ps)
                    else:
                        nc.scalar.copy(out=so[:, nsl], in_=ps)
                nc.sync.dma_start(out=out_d[m * 128:(m + 1) * 128, 0:1024],
                                  in_=so[:, 0:1024])
                nc.gpsimd.dma_start(
                    out=out_d[m * 128:(m + 1) * 128, 1024:2048],
                    in_=so[:, 1024:2048])

            def after_chunk(w, c2):
                for m in range(SUBS * w + 4 * c2, SUBS * w + 4 * (c2 + 1)):
                    c_mtile(m)

            for w in range(N_QW):
                attn_window(1, w, after_chunk=after_chunk)
            stC.close()
            stB.close()
            stA.close()

    with tile.TileContext(nc) as tc:
        for _ in range(reps):
            emit_body(tc)

    nc.compile()
    return nc


def _host_prep(inputs):
    x = np.ascontiguousarray(np.asarray(inputs["x"], dtype=np.float32)[0])  # [S, D]
    tp = np.asarray(inputs["token_positions"]).reshape(-1)[:S]
    Wq = np.asarray(inputs["Wq"], dtype=np.float32)
    Wk = np.asarray(inputs["Wk"], dtype=np.float32)
    Wv = np.asarray(inputs["Wv"], dtype=np.float32)
    Wo = np.asarray(inputs["Wo"], dtype=np.float32)

    # x^T in per-partition-contiguous window-major layout:
    # [p, w, t, s'] = x^T[t*128+p, w*512+s']  ->  [128, S*16]
    xT = np.ascontiguousarray(x.T).astype(BF16)  # [D, S]
    xTw = np.ascontiguousarray(
        xT.reshape(16, 128, S // 512, 512).transpose(1, 2, 0, 3)
    ).reshape(128, S * 16)

    # f32 RoPE tables, replicated across the two 64-row halves
    inv_freq = (10000.0 ** (-np.arange(0, DH, 2, dtype=np.float32) / DH)
                ).astype(np.float32)
    ang = tp.astype(np.float32)[:, None] * inv_freq[None, :]  # [S, 64] f32
    cos = np.cos(ang).astype(np.float32).T  # [64, S]
    sin = np.sin(ang).astype(np.float32).T
    cosF = np.concatenate([cos, cos], axis=0).astype(BF16)  # [128, S]
    sinX = np.concatenate([-sin, sin], axis=0).astype(BF16)
    # half-swap permutation as a matmul lhsT: out[m] = in[(m+64) % 128]
    swp = np.zeros((128, 128), dtype=np.float32)
    swp[np.arange(128), (np.arange(128) + 64) % 128] = 1.0
    swp = swp.astype(BF16)

    # causal mask in scores^T layout: valid iff k <= q  ->  upper triangular
    mask = np.triu(np.ones((128, 128), dtype=np.float32)).astype(BF16)

    perm = np.concatenate([np.arange(0, DH, 2), np.arange(1, DH, 2)])
    in_maps = []
    for c in range(N_CORES):
        rows = slice(c * HPC * DH, (c + 1) * HPC * DH)
        wq_blk = Wq[rows].reshape(HPC, DH, D)[:, perm].reshape(HPC * DH, D)
        wk_blk = Wk[rows].reshape(HPC, DH, D)[:, perm].reshape(HPC * DH, D)
        wv_blk = Wv[rows]
        def _wlay(blk):  # [256, D] -> [p, t, m] contiguous [128, 4096]
            bt = np.ascontiguousarray(blk.T).astype(BF16)  # [D, 256]
            return np.ascontiguousarray(
                bt.reshape(16, 128, HPC * DH).transpose(1, 0, 2)
            ).reshape(128, 16 * HPC * DH)

        in_maps.append({
            "xTw": xTw,
            "wqT": _wlay(wq_blk),
            "wkT": _wlay(wk_blk),
            "wvT": _wlay(wv_blk),
            "woT": np.ascontiguousarray(Wo[:, rows].T).astype(BF16),
            "cosF": cosF,
            "sinX": sinX,
            "swp": swp,
            "mask": mask,
        })
    return in_maps


def get_compiled():
    if "nc" not in _CACHE:
        _CACHE["nc"] = _build()
    return _CACHE["nc"]


def _get_runner(nc):
    """Build (once) a jitted 8-core runner; reused across kernel() calls."""
    if "runner" in _CACHE:
        return _CACHE["runner"]
    import jax
    from jax.sharding import Mesh, PartitionSpec
    from jax.experimental.shard_map import shard_map
    from concourse import bass2jax

    bass2jax.install_neuronx_cc_hook()
    part_name = (nc.partition_id_tensor.name
                 if nc.partition_id_tensor else None)
    in_names, out_names, out_avals, zero_outs = [], [], [], []
    for alloc in nc.m.functions[0].allocations:
        if not isinstance(alloc, mybir.MemoryLocationSet):
            continue
        name = alloc.memorylocations[0].name
        if alloc.kind == "ExternalInput":
            if name != part_name:
                in_names.append(name)
        elif alloc.kind == "ExternalOutput":
            shape = tuple(alloc.tensor_shape)
            dtype = mybir.dt.np(alloc.dtype)
            out_names.append(name)
            out_avals.append(jax.core.ShapedArray(shape, dtype))
            zero_outs.append(np.zeros(shape, dtype))
    n_params = len(in_names)
    all_in_names = list(in_names) + list(out_names)
    if part_name is not None:
        all_in_names = all_in_names + [part_name]

    def _body(*args):
        ins = list(args[:n_params])
        outs = list(args[n_params:])
        operands = ins + outs
        if part_name is not None:
            operands.append(bass2jax.partition_id_tensor())
        outs = list(bass2jax._bass_exec_p.bind(
            *operands,
            out_avals=tuple(out_avals),
            in_names=tuple(all_in_names),
            out_names=tuple(out_names),
            lowering_input_output_aliases=(),
            sim_require_finite=True,
            sim_require_nnan=True,
            nc=nc,
        ))
        return tuple(outs)

    devices = jax.devices()[:N_CORES]
    mesh = Mesh(np.asarray(devices), ("core",))
    nin = n_params + len(out_names)
    sharded = jax.jit(
        shard_map(_body, mesh=mesh,
                  in_specs=(PartitionSpec("core"),) * nin,
                  out_specs=(PartitionSpec("core"),) * len(out_names),
                  check_rep=False),
        keep_unused=True,
    )
    concat_zero = [np.zeros((N_CORES * z.shape[0], *z.shape[1:]), z.dtype)
                   for z in zero_outs]
    _CACHE["runner"] = (sharded, in_names, out_names, concat_zero)
    return _CACHE["runner"]


def kernel(**inputs):
    import jax
    nc = get_compiled()
    in_maps = _host_prep(inputs)
    sharded, in_names, out_names, concat_zero = _get_runner(nc)
    concat_in = [np.concatenate([np.asarray(in_maps[c][nm])
                                 for c in range(N_CORES)], axis=0)
                 for nm in in_names]
    out = sharded(*[jax.device_put(a) for a in concat_in + concat_zero])
    oi = out_names.index("out")
    res = np.asarray(out[oi]).reshape(N_CORES, S, D)
    y = res.astype(np.float32).sum(axis=0)
    return y.reshape(1, S, D)


# revision 43
# speedup vs baseline: 2.9064x; 1.3049x over previous
"""Multi-head self-attention (RoPE, causal) Trainium2 Bass kernel.

Sharding: tensor-parallel over heads. 16 heads / 8 cores = 2 heads per core.
Each core computes Q/K/V projections for its 2 heads, causal attention, and a
partial output projection against its 256-column slice of Wo. The host sums
the 8 partial [S, D] outputs.

All matmuls run in bf16 with fp32 PSUM accumulation. Softmax skips the max
subtraction (scores are O(1) for this problem family; exp stays in fp32
range). The attention*V phase is computed transposed — V tiles are the
stationary operand and P^T streams 512-wide q-chunks — so the PE streams full
free dims instead of 129-wide LDW-bound matmuls, and the output lands
directly in the [dh, q] layout the output projection wants (no transposes).
The softmax denominator comes from an f32 running sum of P^T tiles on the
vector engine, reduced across partitions by gpsimd.partition_all_reduce.
RoPE's even/odd interleave is folded into a row permutation of Wq/Wk, making
the on-device rotation a contiguous rotate-half.
"""
import sys

sys.path.insert(0, "/opt/trn_rl_repo")

import numpy as np
import ml_dtypes

import concourse.bass as bass  # noqa: F401  (registers AP machinery)
import concourse.tile as tile
from concourse import bacc, bass_isa, mybir
from concourse import bass_utils

BF16 = ml_dtypes.bfloat16
S = 4096
D = 2048
DH = 128
N_CORES = 8
HPC = 2  # heads per core
PW = 512  # projection s-window
QW = 1024  # attention q-window
N_QW = S // QW  # 4
SUBS = QW // 128  # 8 q-subtiles per window
N_KT = S // 128  # 32 k-tiles
INV_SQRT_DH = float(1.0 / np.sqrt(128.0))

_CACHE = {}


def _build(reps=1):
    fp32 = mybir.dt.float32
    bf16 = mybir.dt.bfloat16

    nc = bacc.Bacc("TRN2", target_bir_lowering=False, debug=False,
                   num_devices=N_CORES)
    xT_d = nc.dram_tensor("xTw", [128, S * 16], bf16,
                          kind="ExternalInput").ap()
    wq_d = nc.dram_tensor("wqT", [128, 16 * HPC * DH], bf16,
                          kind="ExternalInput").ap()
    wk_d = nc.dram_tensor("wkT", [128, 16 * HPC * DH], bf16,
                          kind="ExternalInput").ap()
    wv_d = nc.dram_tensor("wvT", [128, 16 * HPC * DH], bf16,
                          kind="ExternalInput").ap()
    wo_d = nc.dram_tensor("woT", [HPC * DH, D], bf16, kind="ExternalInput").ap()
    cos_d = nc.dram_tensor("cosF", [128, S], bf16, kind="ExternalInput").ap()
    sin_d = nc.dram_tensor("sinX", [128, S], bf16, kind="ExternalInput").ap()
    swp_d = nc.dram_tensor("swp", [128, 128], bf16, kind="ExternalInput").ap()
    mask_d = nc.dram_tensor("mask", [128, 128], bf16, kind="ExternalInput").ap()
    out_d = nc.dram_tensor("out", [S, D], bf16, kind="ExternalOutput").ap()

    xT_r = xT_d.rearrange("p (w t s) -> p w t s", w=S // PW,
                          t=16)               # [128, 8, 16, 512]
    wq_r = wq_d.rearrange("p (t m) -> p t m", t=16)    # [128, 16, 256]
    wk_r = wk_d.rearrange("p (t m) -> p t m", t=16)
    wv_r = wv_d.rearrange("p (t m) -> p t m", t=16)

    EXP = mybir.ActivationFunctionType.Exp

    from contextlib import ExitStack

    def emit_body(tc):
        with tc.tile_pool(name="persist", bufs=1) as pp, \
             tc.tile_pool(name="ropet", bufs=1) as rtp:
            # ---- persistent tiles + initial loads (spread across engines)
            qt = [pp.tile([128, S], bf16, tag=f"qt{h}", name=f"qt{h}")
                  for h in range(HPC)]
            kt = [pp.tile([128, S], bf16, tag=f"kt{h}", name=f"kt{h}")
                  for h in range(HPC)]
            v_sb = pp.tile([128, N_KT, HPC * DH], bf16, tag="v")
            oc = [[pp.tile([128, QW], bf16, tag=f"oc{h}w{w}", name=f"oc{h}w{w}")
                   for w in range(N_QW)] for h in range(HPC)]
            cos_sb = pp.tile([128, S], bf16, tag="cos")
            sin_sb = pp.tile([128, S], bf16, tag="sin")
            mask_sb = pp.tile([128, 128], bf16, tag="mask")
            swp_sb = pp.tile([128, 128], bf16, tag="swp")

            stA = ExitStack()
            psA = stA.enter_context(
                tc.tile_pool(name="psA", bufs=2, space="PSUM"))
            wqkp = stA.enter_context(tc.tile_pool(name="wqk", bufs=1))
            wq_sb = wqkp.tile([128, 16, HPC * DH], bf16, tag="wq")
            wk_sb = wqkp.tile([128, 16, HPC * DH], bf16, tag="wk")
            stV = ExitStack()
            psV = stV.enter_context(
                tc.tile_pool(name="psV", bufs=2, space="PSUM"))
            wvp = stV.enter_context(tc.tile_pool(name="wvp", bufs=1))
            xw1p = stV.enter_context(tc.tile_pool(name="xw1", bufs=3))
            wv_sb = wvp.tile([128, 16, HPC * DH], bf16, tag="wv")

            nc.sync.dma_start(out=wq_sb[:, 0:8, :], in_=wq_r[:, 0:8, :])
            nc.sync.dma_start(out=wq_sb[:, 8:16, :], in_=wq_r[:, 8:16, :])
            nc.scalar.dma_start(out=wk_sb[:, 0:8, :], in_=wk_r[:, 0:8, :])
            nc.scalar.dma_start(out=wk_sb[:, 8:16, :], in_=wk_r[:, 8:16, :])
            nc.scalar.dma_start(out=wv_sb, in_=wv_r)
            nc.scalar.dma_start(out=swp_sb, in_=swp_d)
            nc.scalar.dma_start(out=cos_sb, in_=cos_d)
            nc.scalar.dma_start(out=sin_sb, in_=sin_d)

            wo_sb = [pp.tile([128, D], bf16, tag=f"wo{t}", name=f"wo{t}")
                     for t in range(HPC)]

            def project_qk(h, xw, sl, pw):
                hs = slice(h * DH, (h + 1) * DH)
                for wsb, dest in ((wq_sb, qt[h]), (wk_sb, kt[h])):
                    ps = psA.tile([128, pw], fp32, tag="qk", name="ps",
                                  padded_shape=[128, PW])
                    for t in range(16):
                        nc.tensor.matmul(ps, wsb[:, t, hs], xw[:, t, :],
                                         start=(t == 0), stop=(t == 15))
                    if h == 0:
                        nc.scalar.copy(out=dest[:, sl], in_=ps)
                    else:
                        nc.vector.tensor_copy(dest[:, sl], ps)
                    # rope in place: dest = dest*cosF + swap(dest)*[-sin;sin]
                    dsl = dest[:, sl]
                    swp = psA.tile([128, pw], fp32, tag="qk", bufs=2,
                                   name="swp", padded_shape=[128, PW])
                    nc.tensor.matmul(swp, swp_sb, dsl, start=True, stop=True)
                    m1 = rtp.tile([128, pw], bf16, tag="m1", name="m1",
                                  padded_shape=[128, PW])
                    m2 = rtp.tile([128, pw], bf16, tag="m2", name="m2",
                                  padded_shape=[128, PW])
                    nc.vector.tensor_mul(m1, dsl, cos_sb[:, sl])
                    nc.vector.tensor_mul(m2, swp, sin_sb[:, sl])
                    nc.vector.tensor_add(dsl, m1, m2)

            # ---------------- A1: head-0 Q/K + all V ----------------
            # first 512-window split into 256 halves so the very first
            # projection starts after ~1.5 MB of DMA; rope tables stream
            # in per-piece slices; mask/wo defer a few windows in
            pieces = [(0, 256), (256, 256)] + [
                (k * PW, PW) for k in range(1, S // PW)]
            for pi, (p0, pw) in enumerate(pieces):
                sl = slice(p0, p0 + pw)
                wi, wo_off = p0 // PW, p0 % PW
                xw = xw1p.tile([128, 16, pw], bf16, tag="xw",
                               name="xw", padded_shape=[128, 16, PW])
                nc.sync.dma_start(
                    out=xw[:, 0:8, :],
                    in_=xT_r[:, wi, 0:8, wo_off:wo_off + pw])
                nc.gpsimd.dma_start(
                    out=xw[:, 8:16, :],
                    in_=xT_r[:, wi, 8:16, wo_off:wo_off + pw])
                if pi == 4:
                    nc.scalar.dma_start(out=mask_sb, in_=mask_d)
                elif pi == 5:
                    for t in range(HPC):
                        nc.scalar.dma_start(
                            out=wo_sb[t], in_=wo_d[t * 128:(t + 1) * 128, :])
                project_qk(0, xw, sl, pw)
                for sub in range(pw // 128):
                    st = p0 // 128 + sub
                    ssl = slice(sub * 128, (sub + 1) * 128)
                    pv = psV.tile([128, HPC * DH], fp32, tag="v")
                    for t in range(16):
                        nc.tensor.matmul(pv, xw[:, t, ssl], wv_sb[:, t, :],
                                         start=(t == 0), stop=(t == 15))
                    nc.scalar.copy(out=v_sb[:, st, :], in_=pv)
            stV.close()

            # ---------------- B machinery ----------------
            stB = ExitStack()
            ptp = stB.enter_context(tc.tile_pool(name="pt", bufs=1))
            psmp = stB.enter_context(tc.tile_pool(name="psm", bufs=2))
            rcpp = stB.enter_context(tc.tile_pool(name="rcp", bufs=3))
            pssc = stB.enter_context(
                tc.tile_pool(name="pssc", bufs=2, space="PSUM"))
            psag = stB.enter_context(
                tc.tile_pool(name="psag", bufs=2, space="PSUM"))

            def attn_scores(h, w):
                q0 = w * QW
                n_j = SUBS * w + SUBS
                par = [2 if w > 0 else 1, 2]  # accumulators in use per chunk
                psum = [[psmp.tile([128, 512], bf16, tag=f"psum{c2}p{p}",
                                   name=f"psum{c2}p{p}")
                         for p in range(par[c2])] for c2 in range(2)]
                nacc = [0, 0]  # contributions so far per chunk
                pts = []
                for j in range(n_j):
                    ksl = slice(j * 128, (j + 1) * 128)
                    c = j - SUBS * w  # >= 0 -> diagonal strip
                    lo = max(0, c) * 128
                    sc = pssc.tile([128, QW], fp32, tag="sc", name="sc")
                    if lo < 512:
                        nc.tensor.matmul(sc[:, lo:512], kt[h][:, ksl],
                                         qt[h][:, q0 + lo:q0 + 512],
                                         start=True, stop=True)
                        nc.tensor.matmul(sc[:, 512:1024], kt[h][:, ksl],
                                         qt[h][:, q0 + 512:q0 + 1024],
                                         start=True, stop=True)
                    else:
                        nc.tensor.matmul(sc[:, lo:1024], kt[h][:, ksl],
                                         qt[h][:, q0 + lo:q0 + 1024],
                                         start=True, stop=True)
                    pt = ptp.tile([128, QW], bf16, tag=f"pt{j}",
                                  name=f"pt{j}")
                    nc.scalar.activation(pt[:, lo:], sc[:, lo:], EXP,
                                         scale=INV_SQRT_DH)
                    if c >= 0:
                        csl = slice(c * 128, (c + 1) * 128)
                        nc.vector.tensor_mul(pt[:, csl], pt[:, csl], mask_sb)
                    # running bf16 denominator sums: per 512-chunk (so
                    # chunk-0's D closes 4 exps early) and parity-split to
                    # halve the serial DVE chain; valid columns only
                    for c2 in range(2):
                        a = max(lo - c2 * 512, 0)
                        if a >= 512:
                            continue
                        acc = psum[c2][nacc[c2] % par[c2]]
                        srcp = pt[:, c2 * 512 + a:(c2 + 1) * 512]
                        if nacc[c2] < par[c2]:
                            assert a == 0 or par[c2] == 1 or nacc[c2] == 0
                            if a == 0:
                                nc.vector.tensor_copy(
                                    acc, pt[:, c2 * 512:(c2 + 1) * 512])
                            else:
                                nc.vector.tensor_copy(acc[:, a:], srcp)
                                nc.vector.memset(acc[:, 0:a], 0.0)
                        else:
                            nc.vector.tensor_add(acc[:, a:], acc[:, a:],
                                                 srcp)
                        nacc[c2] += 1
                    pts.append(pt)
                # denominator: all-partitions sum (f32 internally) -> recip
                rc = rcpp.tile([128, QW], fp32, tag="rc", name="rc")
                for c2 in range(2):
                    cl = slice(c2 * 512, (c2 + 1) * 512)
                    if par[c2] == 2:
                        nc.vector.tensor_add(psum[c2][0], psum[c2][0],
                                             psum[c2][1])
                    nc.gpsimd.partition_all_reduce(
                        rc[:, cl], psum[c2][0], channels=128,
                        reduce_op=bass_isa.ReduceOp.add)
                    nc.vector.reciprocal_approx_fast(rc[:, cl], rc[:, cl])
                return pts, rc

            def attn_av(h, w, pts, rc, after_chunk=None):
                hs2 = slice(h * DH, (h + 1) * DH)
                # attention * V, transposed: aug[dh, q] += V_j^T P_j
                # (diagonal tiles stream only their causally-valid columns)
                for c2 in range(2):
                    jn = SUBS * w + 4 * (c2 + 1)
                    chunk0 = c2 * 512
                    aug = psag.tile([128, 512], fp32, tag="aug", name="aug")
                    for j in range(jn):
                        jj = j - SUBS * w - 4 * c2  # >=0: diagonal in chunk
                        a = max(0, jj) * 128
                        nc.tensor.matmul(
                            aug[:, a:512], v_sb[:, j, hs2],
                            pts[j][:, chunk0 + a:chunk0 + 512],
                            start=(j == 0), stop=(j == jn - 1))
                    nc.vector.tensor_mul(
                        oc[h][w][:, chunk0:chunk0 + 512], aug,
                        rc[:, chunk0:chunk0 + 512])
                    if after_chunk is not None:
                        after_chunk(w, c2)

            def attn_window(h, w, after_chunk=None):
                pts, rc = attn_scores(h, w)
                attn_av(h, w, pts, rc, after_chunk=after_chunk)

            # ------- A2 (head-1 Q/K, quarter windows) ∥ B head-0 -------
            PW2 = 256
            stX2 = ExitStack()
            xw2p = stX2.enter_context(tc.tile_pool(name="xw2", bufs=2))
            n_q = S // PW2  # 16 quarter windows
            for w in range(N_QW):
                attn_window(0, w)
                for q in range(n_q // N_QW * w, n_q // N_QW * (w + 1)):
                    sl = slice(q * PW2, (q + 1) * PW2)
                    ho = (q % 2) * PW2
                    xw = xw2p.tile([128, 16, PW2], bf16, tag="xw2",
                                   name="xw2")
                    nc.sync.dma_start(out=xw[:, 0:8, :],
                                      in_=xT_r[:, q // 2, 0:8, ho:ho + PW2])
                    nc.sync.dma_start(
                        out=xw[:, 8:16, :],
                        in_=xT_r[:, q // 2, 8:16, ho:ho + PW2])
                    project_qk(1, xw, sl, PW2)
            stX2.close()

            # ---------------- B head-1 ∥ C ----------------
            stC = ExitStack()
            cst = stC.enter_context(tc.tile_pool(name="cst", bufs=4))

            def c_mtile(m):
                msl = slice((m % SUBS) * 128, (m % SUBS + 1) * 128)
                last = m >= S // 128 - SUBS
                so = cst.tile([128, D], bf16, tag="so", name="so")
                for nw in range(D // 512):
                    nsl = slice(nw * 512, (nw + 1) * 512)
                    ps = psA.tile([128, 512], fp32, tag="qk", name="cps")
                    for t in range(HPC):
                        nc.tensor.matmul(ps, oc[t][m // SUBS][:, msl],
                                         wo_sb[t][:, nsl],
                                         start=(t == 0), stop=(t == HPC - 1))
                    if nw % 2 == 0 and not last:
                        nc.vector.tensor_copy(so[:, nsl], ps)
                    else:
                        nc.scalar.copy(out=so[:, nsl], in_=ps)
                nc.sync.dma_start(out=out_d[m * 128:(m + 1) * 128, 0:1024],
                                  in_=so[:, 0:1024])
                nc.sync.dma_start(
                    out=out_d[m * 128:(m + 1) * 128, 1024:2048],
                    in_=so[:, 1024:2048])

            def after_chunk(w, c2):
                for m in range(SUBS * w + 4 * c2, SUBS * w + 4 * (c2 + 1)):
                    c_mtile(m)

            for w in range(N_QW):
                attn_window(1, w, after_chunk=after_chunk)
            stC.close()
            stB.close()
            stA.close()

    with tile.TileContext(nc) as tc:
        for _ in range(reps):
            emit_body(tc)

    nc.compile()
    return nc


def _host_prep(inputs):
    x = np.ascontiguousarray(np.asarray(inputs["x"], dtype=np.float32)[0])  # [S, D]
    tp = np.asarray(inputs["token_positions"]).reshape(-1)[:S]
    Wq = np.asarray(inputs["Wq"], dtype=np.float32)
    Wk = np.asarray(inputs["Wk"], dtype=np.float32)
    Wv = np.asarray(inputs["Wv"], dtype=np.float32)
    Wo = np.asarray(inputs["Wo"], dtype=np.float32)

    # x^T in per-partition-contiguous window-major layout:
    # [p, w, t, s'] = x^T[t*128+p, w*512+s']  ->  [128, S*16]
    xT = np.ascontiguousarray(x.T).astype(BF16)  # [D, S]
    xTw = np.ascontiguousarray(
        xT.reshape(16, 128, S // 512, 512).transpose(1, 2, 0, 3)
    ).reshape(128, S * 16)

    # f32 RoPE tables, replicated across the two 64-row halves
    inv_freq = (10000.0 ** (-np.arange(0, DH, 2, dtype=np.float32) / DH)
                ).astype(np.float32)
    ang = tp.astype(np.float32)[:, None] * inv_freq[None, :]  # [S, 64] f32
    cos = np.cos(ang).astype(np.float32).T  # [64, S]
    sin = np.sin(ang).astype(np.float32).T
    cosF = np.concatenate([cos, cos], axis=0).astype(BF16)  # [128, S]
    sinX = np.concatenate([-sin, sin], axis=0).astype(BF16)
    # half-swap permutation as a matmul lhsT: out[m] = in[(m+64) % 128]
    swp = np.zeros((128, 128), dtype=np.float32)
    swp[np.arange(128), (np.arange(128) + 64) % 128] = 1.0
    swp = swp.astype(BF16)

    # causal mask in scores^T layout: valid iff k <= q  ->  upper triangular
    mask = np.triu(np.ones((128, 128), dtype=np.float32)).astype(BF16)

    perm = np.concatenate([np.arange(0, DH, 2), np.arange(1, DH, 2)])
    in_maps = []
    for c in range(N_CORES):
        rows = slice(c * HPC * DH, (c + 1) * HPC * DH)
        wq_blk = Wq[rows].reshape(HPC, DH, D)[:, perm].reshape(HPC * DH, D)
        wk_blk = Wk[rows].reshape(HPC, DH, D)[:, perm].reshape(HPC * DH, D)
        wv_blk = Wv[rows]
        def _wlay(blk):  # [256, D] -> [p, t, m] contiguous [128, 4096]
            bt = np.ascontiguousarray(blk.T).astype(BF16)  # [D, 256]
            return np.ascontiguousarray(
                bt.reshape(16, 128, HPC * DH).transpose(1, 0, 2)
            ).reshape(128, 16 * HPC * DH)

        in_maps.append({
            "xTw": xTw,
            "wqT": _wlay(wq_blk),
            "wkT": _wlay(wk_blk),
            "wvT": _wlay(wv_blk),
            "woT": np.ascontiguousarray(Wo[:, rows].T).astype(BF16),
            "cosF": cosF,
            "sinX": sinX,
            "swp": swp,
            "mask": mask,
        })
    return in_maps


def get_compiled():
    if "nc" not in _CACHE:
        _CACHE["nc"] = _build()
    return _CACHE["nc"]


def _get_runner(nc):
    """Build (once) a jitted 8-core runner; reused across kernel() calls."""
    if "runner" in _CACHE:
        return _CACHE["runner"]
    import jax
    from jax.sharding import Mesh, PartitionSpec
    from jax.experimental.shard_map import shard_map
    from concourse import bass2jax

    bass2jax.install_neuronx_cc_hook()
    part_name = (nc.partition_id_tensor.name
                 if nc.partition_id_tensor else None)
    in_names, out_names, out_avals, zero_outs = [], [], [], []
    for alloc in nc.m.functions[0].allocations:
        if not isinstance(alloc, mybir.MemoryLocationSet):
            continue
        name = alloc.memorylocations[0].name
        if alloc.kind == "ExternalInput":
            if name != part_name:
                in_names.append(name)
        elif alloc.kind == "ExternalOutput":
            shape = tuple(alloc.tensor_shape)
            dtype = mybir.dt.np(alloc.dtype)
            out_names.append(name)
            out_avals.append(jax.core.ShapedArray(shape, dtype))
            zero_outs.append(np.zeros(shape, dtype))
    n_params = len(in_names)
    all_in_names = list(in_names) + list(out_names)
    if part_name is not None:
        all_in_names = all_in_names + [part_name]

    def _body(*args):
        ins = list(args[:n_params])
        outs = list(args[n_params:])
        operands = ins + outs
        if part_name is not None:
            operands.append(bass2jax.partition_id_tensor())
        outs = list(bass2jax._bass_exec_p.bind(
            *operands,
            out_avals=tuple(out_avals),
            in_names=tuple(all_in_names),
            out_names=tuple(out_names),
            lowering_input_output_aliases=(),
            sim_require_finite=True,
            sim_require_nnan=True,
            nc=nc,
        ))
        return tuple(outs)

    devices = jax.devices()[:N_CORES]
    mesh = Mesh(np.asarray(devices), ("core",))
    nin = n_params + len(out_names)
    sharded = jax.jit(
        shard_map(_body, mesh=mesh,
                  in_specs=(PartitionSpec("core"),) * nin,
                  out_specs=(PartitionSpec("core"),) * len(out_names),
                  check_rep=False),
        keep_unused=True,
    )
    concat_zero = [np.zeros((N_CORES * z.shape[0], *z.shape[1:]), z.dtype)
                   for z in zero_outs]
    _CACHE["runner"] = (sharded, in_names, out_names, concat_zero)
    return _CACHE["runner"]


def kernel(**inputs):
    import jax
    nc = get_compiled()
    in_maps = _host_prep(inputs)
    sharded, in_names, out_names, concat_zero = _get_runner(nc)
    concat_in = [np.concatenate([np.asarray(in_maps[c][nm])
                                 for c in range(N_CORES)], axis=0)
                 for nm in in_names]
    out = sharded(*[jax.device_put(a) for a in concat_in + concat_zero])
    oi = out_names.index("out")
    res = np.asarray(out[oi]).reshape(N_CORES, S, D)
    y = res.astype(np.float32).sum(axis=0)
    return y.reshape(1, S, D)


# revision 46
# speedup vs baseline: 4.9027x; 1.6868x over previous
"""Multi-head self-attention (RoPE, causal) Trainium2 Bass kernel.

Sharding: tensor-parallel over heads. 16 heads / 8 cores = 2 heads per core.
Each core computes Q/K/V projections for its 2 heads, causal attention, and a
partial output projection against its 256-column slice of Wo. The host sums
the 8 partial [S, D] outputs.

All matmuls run in bf16 with fp32 PSUM accumulation. Softmax skips the max
subtraction (scores are O(1) for this problem family; exp stays in fp32
range). The attention*V phase is computed transposed — V tiles are the
stationary operand and P^T streams 512-wide q-chunks — so the PE streams full
free dims instead of 129-wide LDW-bound matmuls, and the output lands
directly in the [dh, q] layout the output projection wants (no transposes).
The softmax denominator comes from an f32 running sum of P^T tiles on the
vector engine, reduced across partitions by gpsimd.partition_all_reduce.
RoPE's even/odd interleave is folded into a row permutation of Wq/Wk, making
the on-device rotation a contiguous rotate-half.
"""
import sys

sys.path.insert(0, "/opt/trn_rl_repo")

import numpy as np
import ml_dtypes

import concourse.bass as bass  # noqa: F401  (registers AP machinery)
import concourse.tile as tile
from concourse import bacc, bass_isa, mybir
from concourse import bass_utils

BF16 = ml_dtypes.bfloat16
S = 4096
D = 2048
DH = 128
N_CORES = 8
HPC = 2  # heads per core
PW = 512  # projection s-window
QW = 1024  # attention q-window
N_QW = S // QW  # 4
SUBS = QW // 128  # 8 q-subtiles per window
N_KT = S // 128  # 32 k-tiles
INV_SQRT_DH = float(1.0 / np.sqrt(128.0))

_CACHE = {}


def _build(reps=1):
    fp32 = mybir.dt.float32
    bf16 = mybir.dt.bfloat16

    nc = bacc.Bacc("TRN2", target_bir_lowering=False, debug=False,
                   num_devices=N_CORES)
    xT_d = nc.dram_tensor("xTw", [128, S * 16], bf16,
                          kind="ExternalInput").ap()
    wq_d = nc.dram_tensor("wqT", [128, 16 * HPC * DH], bf16,
                          kind="ExternalInput").ap()
    wk_d = nc.dram_tensor("wkT", [128, 16 * HPC * DH], bf16,
                          kind="ExternalInput").ap()
    wv_d = nc.dram_tensor("wvT", [128, 16 * HPC * DH], bf16,
                          kind="ExternalInput").ap()
    wo_d = nc.dram_tensor("woT", [HPC * DH, D], bf16, kind="ExternalInput").ap()
    cos_d = nc.dram_tensor("cosF", [128, S], bf16, kind="ExternalInput").ap()
    sin_d = nc.dram_tensor("sinX", [128, S], bf16, kind="ExternalInput").ap()
    swp_d = nc.dram_tensor("swp", [128, 128], bf16, kind="ExternalInput").ap()
    mask_d = nc.dram_tensor("mask", [128, 128], bf16, kind="ExternalInput").ap()
    out_d = nc.dram_tensor("out", [S, D], bf16, kind="ExternalOutput").ap()

    xT_r = xT_d.rearrange("p (w t s) -> p w t s", w=S // PW,
                          t=16)               # [128, 8, 16, 512]
    wq_r = wq_d.rearrange("p (t m) -> p t m", t=16)    # [128, 16, 256]
    wk_r = wk_d.rearrange("p (t m) -> p t m", t=16)
    wv_r = wv_d.rearrange("p (t m) -> p t m", t=16)

    EXP = mybir.ActivationFunctionType.Exp

    from contextlib import ExitStack

    def emit_body(tc):
        with tc.tile_pool(name="persist", bufs=1) as pp, \
             tc.tile_pool(name="ropet", bufs=1) as rtp:
            # ---- persistent tiles + initial loads (spread across engines)
            qt = [pp.tile([128, S], bf16, tag=f"qt{h}", name=f"qt{h}")
                  for h in range(HPC)]
            kt = [pp.tile([128, S], bf16, tag=f"kt{h}", name=f"kt{h}")
                  for h in range(HPC)]
            v_sb = pp.tile([128, N_KT, HPC * DH], bf16, tag="v")
            oc = [[pp.tile([128, QW], bf16, tag=f"oc{h}w{w}", name=f"oc{h}w{w}")
                   for w in range(N_QW)] for h in range(HPC)]
            cos_sb = pp.tile([128, S], bf16, tag="cos")
            sin_sb = pp.tile([128, S], bf16, tag="sin")
            mask_sb = pp.tile([128, 128], bf16, tag="mask")
            swp_sb = pp.tile([128, 128], bf16, tag="swp")

            stA = ExitStack()
            psA = stA.enter_context(
                tc.tile_pool(name="psA", bufs=2, space="PSUM"))
            wqkp = stA.enter_context(tc.tile_pool(name="wqk", bufs=1))
            wq_sb = wqkp.tile([128, 16, HPC * DH], bf16, tag="wq")
            wk_sb = wqkp.tile([128, 16, HPC * DH], bf16, tag="wk")
            stV = ExitStack()
            psV = stV.enter_context(
                tc.tile_pool(name="psV", bufs=2, space="PSUM"))
            wvp = stV.enter_context(tc.tile_pool(name="wvp", bufs=1))
            xw1p = stV.enter_context(tc.tile_pool(name="xw1", bufs=3))
            wv_sb = wvp.tile([128, 16, HPC * DH], bf16, tag="wv")

            nc.sync.dma_start(out=wq_sb[:, 0:8, :], in_=wq_r[:, 0:8, :])
            nc.sync.dma_start(out=wq_sb[:, 8:16, :], in_=wq_r[:, 8:16, :])
            nc.scalar.dma_start(out=wk_sb[:, 0:8, :], in_=wk_r[:, 0:8, :])
            nc.scalar.dma_start(out=wk_sb[:, 8:16, :], in_=wk_r[:, 8:16, :])
            nc.scalar.dma_start(out=wv_sb, in_=wv_r)
            nc.scalar.dma_start(out=swp_sb, in_=swp_d)

            wo_sb = [pp.tile([128, D], bf16, tag=f"wo{t}", name=f"wo{t}")
                     for t in range(HPC)]

            def project_qk(h, xw, sl, pw):
                hs = slice(h * DH, (h + 1) * DH)
                for wsb, dest in ((wq_sb, qt[h]), (wk_sb, kt[h])):
                    ps = psA.tile([128, pw], fp32, tag="qk", name="ps",
                                  padded_shape=[128, PW])
                    for t in range(16):
                        nc.tensor.matmul(ps, wsb[:, t, hs], xw[:, t, :],
                                         start=(t == 0), stop=(t == 15))
                    if h == 0:
                        nc.scalar.copy(out=dest[:, sl], in_=ps)
                    else:
                        nc.vector.tensor_copy(dest[:, sl], ps)
                    # rope in place: dest = dest*cosF + swap(dest)*[-sin;sin]
                    dsl = dest[:, sl]
                    swp = psA.tile([128, pw], fp32, tag="qk", bufs=2,
                                   name="swp", padded_shape=[128, PW])
                    nc.tensor.matmul(swp, swp_sb, dsl, start=True, stop=True)
                    m1 = rtp.tile([128, pw], bf16, tag="m1", name="m1",
                                  padded_shape=[128, PW])
                    m2 = rtp.tile([128, pw], bf16, tag="m2", name="m2",
                                  padded_shape=[128, PW])
                    nc.vector.tensor_mul(m1, dsl, cos_sb[:, sl])
                    nc.vector.tensor_mul(m2, swp, sin_sb[:, sl])
                    nc.vector.tensor_add(dsl, m1, m2)

            # ---------------- A1: head-0 Q/K + all V ----------------
            # first 512-window split into 256 halves so the very first
            # projection starts after ~1.5 MB of DMA; rope tables stream
            # in per-piece slices; mask/wo defer a few windows in
            pieces = [(0, 256), (256, 256)] + [
                (k * PW, PW) for k in range(1, S // PW)]
            for pi, (p0, pw) in enumerate(pieces):
                sl = slice(p0, p0 + pw)
                wi, wo_off = p0 // PW, p0 % PW
                xw = xw1p.tile([128, 16, pw], bf16, tag="xw",
                               name="xw", padded_shape=[128, 16, PW])
                nc.sync.dma_start(
                    out=xw[:, 0:8, :],
                    in_=xT_r[:, wi, 0:8, wo_off:wo_off + pw])
                nc.gpsimd.dma_start(
                    out=xw[:, 8:16, :],
                    in_=xT_r[:, wi, 8:16, wo_off:wo_off + pw])
                if pi == 0:
                    nc.gpsimd.dma_start(out=cos_sb, in_=cos_d)
                    nc.gpsimd.dma_start(out=sin_sb, in_=sin_d)
                elif pi == 4:
                    nc.scalar.dma_start(out=mask_sb, in_=mask_d)
                elif pi == 5:
                    for t in range(HPC):
                        nc.scalar.dma_start(
                            out=wo_sb[t], in_=wo_d[t * 128:(t + 1) * 128, :])
                project_qk(0, xw, sl, pw)
                for sub in range(pw // 128):
                    st = p0 // 128 + sub
                    ssl = slice(sub * 128, (sub + 1) * 128)
                    pv = psV.tile([128, HPC * DH], fp32, tag="v")
                    for t in range(16):
                        nc.tensor.matmul(pv, xw[:, t, ssl], wv_sb[:, t, :],
                                         start=(t == 0), stop=(t == 15))
                    nc.scalar.copy(out=v_sb[:, st, :], in_=pv)
            stV.close()

            # ---------------- B machinery ----------------
            stB = ExitStack()
            ptp = stB.enter_context(tc.tile_pool(name="pt", bufs=1))
            psmp = stB.enter_context(tc.tile_pool(name="psm", bufs=2))
            rcpp = stB.enter_context(tc.tile_pool(name="rcp", bufs=3))
            pssc = stB.enter_context(
                tc.tile_pool(name="pssc", bufs=2, space="PSUM"))
            psag = stB.enter_context(
                tc.tile_pool(name="psag", bufs=2, space="PSUM"))

            def attn_scores(h, w):
                q0 = w * QW
                n_j = SUBS * w + SUBS
                par = [2 if w > 0 else 1, 2]  # accumulators in use per chunk
                psum = [[psmp.tile([128, 512], bf16, tag=f"psum{c2}p{p}",
                                   name=f"psum{c2}p{p}")
                         for p in range(par[c2])] for c2 in range(2)]
                nacc = [0, 0]  # contributions so far per chunk
                pts = []
                for j in range(n_j):
                    ksl = slice(j * 128, (j + 1) * 128)
                    c = j - SUBS * w  # >= 0 -> diagonal strip
                    lo = max(0, c) * 128
                    sc = pssc.tile([128, QW], fp32, tag="sc", name="sc")
                    if lo < 512:
                        nc.tensor.matmul(sc[:, lo:512], kt[h][:, ksl],
                                         qt[h][:, q0 + lo:q0 + 512],
                                         start=True, stop=True)
                        nc.tensor.matmul(sc[:, 512:1024], kt[h][:, ksl],
                                         qt[h][:, q0 + 512:q0 + 1024],
                                         start=True, stop=True)
                    else:
                        nc.tensor.matmul(sc[:, lo:1024], kt[h][:, ksl],
                                         qt[h][:, q0 + lo:q0 + 1024],
                                         start=True, stop=True)
                    pt = ptp.tile([128, QW], bf16, tag=f"pt{j}",
                                  name=f"pt{j}")
                    nc.scalar.activation(pt[:, lo:], sc[:, lo:], EXP,
                                         scale=INV_SQRT_DH)
                    if c >= 0:
                        csl = slice(c * 128, (c + 1) * 128)
                        nc.vector.tensor_mul(pt[:, csl], pt[:, csl], mask_sb)
                    # running bf16 denominator sums: per 512-chunk (so
                    # chunk-0's D closes 4 exps early) and parity-split to
                    # halve the serial DVE chain; valid columns only
                    for c2 in range(2):
                        a = max(lo - c2 * 512, 0)
                        if a >= 512:
                            continue
                        acc = psum[c2][nacc[c2] % par[c2]]
                        srcp = pt[:, c2 * 512 + a:(c2 + 1) * 512]
                        if nacc[c2] < par[c2]:
                            assert a == 0 or par[c2] == 1 or nacc[c2] == 0
                            if a == 0:
                                nc.vector.tensor_copy(
                                    acc, pt[:, c2 * 512:(c2 + 1) * 512])
                            else:
                                nc.vector.tensor_copy(acc[:, a:], srcp)
                                nc.vector.memset(acc[:, 0:a], 0.0)
                        else:
                            nc.vector.tensor_add(acc[:, a:], acc[:, a:],
                                                 srcp)
                        nacc[c2] += 1
                    pts.append(pt)
                # denominator: all-partitions sum (f32 internally) -> recip
                rc = rcpp.tile([128, QW], fp32, tag="rc", name="rc")
                for c2 in range(2):
                    cl = slice(c2 * 512, (c2 + 1) * 512)
                    if par[c2] == 2:
                        nc.vector.tensor_add(psum[c2][0], psum[c2][0],
                                             psum[c2][1])
                    nc.gpsimd.partition_all_reduce(
                        rc[:, cl], psum[c2][0], channels=128,
                        reduce_op=bass_isa.ReduceOp.add)
                    nc.vector.reciprocal_approx_fast(rc[:, cl], rc[:, cl])
                return pts, rc

            def attn_av(h, w, pts, rc, after_chunk=None):
                hs2 = slice(h * DH, (h + 1) * DH)
                # attention * V, transposed: aug[dh, q] += V_j^T P_j
                # (diagonal tiles stream only their causally-valid columns)
                for c2 in range(2):
                    jn = SUBS * w + 4 * (c2 + 1)
                    chunk0 = c2 * 512
                    aug = psag.tile([128, 512], fp32, tag="aug", name="aug")
                    for j in range(jn):
                        jj = j - SUBS * w - 4 * c2  # >=0: diagonal in chunk
                        a = max(0, jj) * 128
                        nc.tensor.matmul(
                            aug[:, a:512], v_sb[:, j, hs2],
                            pts[j][:, chunk0 + a:chunk0 + 512],
                            start=(j == 0), stop=(j == jn - 1))
                    nc.vector.tensor_mul(
                        oc[h][w][:, chunk0:chunk0 + 512], aug,
                        rc[:, chunk0:chunk0 + 512])
                    if after_chunk is not None:
                        after_chunk(w, c2)

            def attn_window(h, w, after_chunk=None):
                pts, rc = attn_scores(h, w)
                attn_av(h, w, pts, rc, after_chunk=after_chunk)

            # ------- A2 (head-1 Q/K, quarter windows) ∥ B head-0 -------
            PW2 = 256
            stX2 = ExitStack()
            xw2p = stX2.enter_context(tc.tile_pool(name="xw2", bufs=2))
            n_q = S // PW2  # 16 quarter windows
            for w in range(N_QW):
                attn_window(0, w)
                for q in range(n_q // N_QW * w, n_q // N_QW * (w + 1)):
                    sl = slice(q * PW2, (q + 1) * PW2)
                    ho = (q % 2) * PW2
                    xw = xw2p.tile([128, 16, PW2], bf16, tag="xw2",
                                   name="xw2")
                    nc.sync.dma_start(out=xw[:, 0:8, :],
                                      in_=xT_r[:, q // 2, 0:8, ho:ho + PW2])
                    nc.sync.dma_start(
                        out=xw[:, 8:16, :],
                        in_=xT_r[:, q // 2, 8:16, ho:ho + PW2])
                    project_qk(1, xw, sl, PW2)
            stX2.close()

            # ---------------- B head-1 ∥ C ----------------
            stC = ExitStack()
            cst = stC.enter_context(tc.tile_pool(name="cst", bufs=4))

            def c_mtile(m):
                msl = slice((m % SUBS) * 128, (m % SUBS + 1) * 128)
                last = m >= S // 128 - SUBS
                so = cst.tile([128, D], bf16, tag="so", name="so")
                for nw in range(D // 512):
                    nsl = slice(nw * 512, (nw + 1) * 512)
                    ps = psA.tile([128, 512], fp32, tag="qk", name="cps")
                    for t in range(HPC):
                        nc.tensor.matmul(ps, oc[t][m // SUBS][:, msl],
                                         wo_sb[t][:, nsl],
                                         start=(t == 0), stop=(t == HPC - 1))
                    if nw % 2 == 0 and not last:
                        nc.vector.tensor_copy(so[:, nsl], ps)
                    else:
                        nc.scalar.copy(out=so[:, nsl], in_=ps)
                nc.sync.dma_start(out=out_d[m * 128:(m + 1) * 128, 0:1024],
                                  in_=so[:, 0:1024])
                nc.sync.dma_start(
                    out=out_d[m * 128:(m + 1) * 128, 1024:2048],
                    in_=so[:, 1024:2048])

            def after_chunk(w, c2):
                for m in range(SUBS * w + 4 * c2, SUBS * w + 4 * (c2 + 1)):
                    c_mtile(m)

            for w in range(N_QW):
                attn_window(1, w, after_chunk=after_chunk)
            stC.close()
            stB.close()
            stA.close()

    with tile.TileContext(nc) as tc:
        for _ in range(reps):
            emit_body(tc)

    nc.compile()
    return nc


def _host_prep(inputs):
    x = np.ascontiguousarray(np.asarray(inputs["x"], dtype=np.float32)[0])  # [S, D]
    tp = np.asarray(inputs["token_positions"]).reshape(-1)[:S]
    Wq = np.asarray(inputs["Wq"], dtype=np.float32)
    Wk = np.asarray(inputs["Wk"], dtype=np.float32)
    Wv = np.asarray(inputs["Wv"], dtype=np.float32)
    Wo = np.asarray(inputs["Wo"], dtype=np.float32)

    # x^T in per-partition-contiguous window-major layout:
    # [p, w, t, s'] = x^T[t*128+p, w*512+s']  ->  [128, S*16]
    xT = np.ascontiguousarray(x.T).astype(BF16)  # [D, S]
    xTw = np.ascontiguousarray(
        xT.reshape(16, 128, S // 512, 512).transpose(1, 2, 0, 3)
    ).reshape(128, S * 16)

    # f32 RoPE tables, replicated across the two 64-row halves
    inv_freq = (10000.0 ** (-np.arange(0, DH, 2, dtype=np.float32) / DH)
                ).astype(np.float32)
    ang = tp.astype(np.float32)[:, None] * inv_freq[None, :]  # [S, 64] f32
    cos = np.cos(ang).astype(np.float32).T  # [64, S]
    sin = np.sin(ang).astype(np.float32).T
    cosF = np.concatenate([cos, cos], axis=0).astype(BF16)  # [128, S]
    sinX = np.concatenate([-sin, sin], axis=0).astype(BF16)
    # half-swap permutation as a matmul lhsT: out[m] = in[(m+64) % 128]
    swp = np.zeros((128, 128), dtype=np.float32)
    swp[np.arange(128), (np.arange(128) + 64) % 128] = 1.0
    swp = swp.astype(BF16)

    # causal mask in scores^T layout: valid iff k <= q  ->  upper triangular
    mask = np.triu(np.ones((128, 128), dtype=np.float32)).astype(BF16)

    perm = np.concatenate([np.arange(0, DH, 2), np.arange(1, DH, 2)])
    in_maps = []
    for c in range(N_CORES):
        rows = slice(c * HPC * DH, (c + 1) * HPC * DH)
        wq_blk = Wq[rows].reshape(HPC, DH, D)[:, perm].reshape(HPC * DH, D)
        wk_blk = Wk[rows].reshape(HPC, DH, D)[:, perm].reshape(HPC * DH, D)
        wv_blk = Wv[rows]
        def _wlay(blk):  # [256, D] -> [p, t, m] contiguous [128, 4096]
            bt = np.ascontiguousarray(blk.T).astype(BF16)  # [D, 256]
            return np.ascontiguousarray(
                bt.reshape(16, 128, HPC * DH).transpose(1, 0, 2)
            ).reshape(128, 16 * HPC * DH)

        in_maps.append({
            "xTw": xTw,
            "wqT": _wlay(wq_blk),
            "wkT": _wlay(wk_blk),
            "wvT": _wlay(wv_blk),
            "woT": np.ascontiguousarray(Wo[:, rows].T).astype(BF16),
            "cosF": cosF,
            "sinX": sinX,
            "swp": swp,
            "mask": mask,
        })
    return in_maps


def get_compiled():
    if "nc" not in _CACHE:
        _CACHE["nc"] = _build()
    return _CACHE["nc"]


def _get_runner(nc):
    """Build (once) a jitted 8-core runner; reused across kernel() calls."""
    if "runner" in _CACHE:
        return _CACHE["runner"]
    import jax
    from jax.sharding import Mesh, PartitionSpec
    from jax.experimental.shard_map import shard_map
    from concourse import bass2jax

    bass2jax.install_neuronx_cc_hook()
    part_name = (nc.partition_id_tensor.name
                 if nc.partition_id_tensor else None)
    in_names, out_names, out_avals, zero_outs = [], [], [], []
    for alloc in nc.m.functions[0].allocations:
        if not isinstance(alloc, mybir.MemoryLocationSet):
            continue
        name = alloc.memorylocations[0].name
        if alloc.kind == "ExternalInput":
            if name != part_name:
                in_names.append(name)
        elif alloc.kind == "ExternalOutput":
            shape = tuple(alloc.tensor_shape)
            dtype = mybir.dt.np(alloc.dtype)
            out_names.append(name)
            out_avals.append(jax.core.ShapedArray(shape, dtype))
            zero_outs.append(np.zeros(shape, dtype))
    n_params = len(in_names)
    all_in_names = list(in_names) + list(out_names)
    if part_name is not None:
        all_in_names = all_in_names + [part_name]

    def _body(*args):
        ins = list(args[:n_params])
        outs = list(args[n_params:])
        operands = ins + outs
        if part_name is not None:
            operands.append(bass2jax.partition_id_tensor())
        outs = list(bass2jax._bass_exec_p.bind(
            *operands,
            out_avals=tuple(out_avals),
            in_names=tuple(all_in_names),
            out_names=tuple(out_names),
            lowering_input_output_aliases=(),
            sim_require_finite=True,
            sim_require_nnan=True,
            nc=nc,
        ))
        return tuple(outs)

    devices = jax.devices()[:N_CORES]
    mesh = Mesh(np.asarray(devices), ("core",))
    nin = n_params + len(out_names)
    sharded = jax.jit(
        shard_map(_body, mesh=mesh,
                  in_specs=(PartitionSpec("core"),) * nin,
                  out_specs=(PartitionSpec("core"),) * len(out_names),
                  check_rep=False),
        keep_unused=True,
    )
    concat_zero = [np.zeros((N_CORES * z.shape[0], *z.shape[1:]), z.dtype)
                   for z in zero_outs]
    _CACHE["runner"] = (sharded, in_names, out_names, concat_zero)
    return _CACHE["runner"]


def kernel(**inputs):
    import jax
    nc = get_compiled()
    in_maps = _host_prep(inputs)
    sharded, in_names, out_names, concat_zero = _get_runner(nc)
    concat_in = [np.concatenate([np.asarray(in_maps[c][nm])
                                 for c in range(N_CORES)], axis=0)
                 for nm in in_names]
    out = sharded(*[jax.device_put(a) for a in concat_in + concat_zero])
    oi = out_names.index("out")
    res = np.asarray(out[oi]).reshape(N_CORES, S, D)
    y = res.astype(np.float32).sum(axis=0)
    return y.reshape(1, S, D)
